# revision 2
# baseline (speedup 1.0000x reference)
"""Trainium2 Bass kernel for a dense transformer encoder block — fp8 DoubleRow.

Sharding (8 cores): sequence-parallel. Core c handles batch b = c//4 and the
512-token query slice q0 = (c%4)*512; K/V are computed for the full batch on
each core.

Datapath: fp8 e4m3 with DoubleRow matmuls (0.5 cyc/row, 2x contraction packed
into the free dim -> 4x the fp16 MAC rate at K=128). The LxL mask is applied
by an extra DoubleRow matmul (-192*I stationary x binary mask moving)
accumulated straight into the scores PSUM; exp(s-2) then goes PSUM->fp8 on the
ACT engine (the -2 shift keeps e4m3 from overflowing and cancels in the
softmax normalization). Scores contract dh=64 as [32,2] (K/Q folded via
partition-shifted DMAs); AV contracts k-token chunk PAIRS as [128,2] with V
laid out 65-wide per head, read through a 128-wide overlapping AP. The FFN
runs 3-term hi/lo fp8 (ah@bh + al@bh + ah@bl, lo-weights in e5m2).

Schedule: V projection runs first under the initial DMA shadow (V copies on
DVE); then K/Q projection + fold stream one head-pair ahead of a
scores->exp->AV software pipeline (AV lags exp by one head), so the ACT
engine's exp stream - the critical resource - starts early and never waits.
PSUM accumulation groups never interleave with another group's start in the
same bank (hardware lazily zeroes the whole 2KB bank on start_tensor_calc).
"""

import sys
from contextlib import ExitStack

import numpy as np

for _p in ("/opt/trn_rl_repo", "/opt/pypackages"):
    if _p not in sys.path:
        sys.path.append(_p)

import ml_dtypes  # noqa: E402
import concourse.bass as bass  # noqa: E402
import concourse.tile as tile  # noqa: E402
from concourse import bacc, mybir  # noqa: E402
from concourse.masks import make_identity  # noqa: E402

F32 = mybir.dt.float32
F16 = mybir.dt.float16
E4 = mybir.dt.float8e4
E5 = mybir.dt.float8e5
E4NP = ml_dtypes.float8_e4m3
E5NP = ml_dtypes.float8_e5m2
AF = mybir.ActivationFunctionType
ALU = mybir.AluOpType
PM = mybir.MatmulPerfMode
DR = PM.DoubleRow

P = 128
DH = 64
LN_EPS = 1e-5

FULL_CFG = dict(B=2, L=2048, D=1024, H=16, DFF=4096, NCORES=8)


def build_bass(cfg):
    B, L, D, H, DFF = cfg["B"], cfg["L"], cfg["D"], cfg["H"], cfg["DFF"]
    NCORES = cfg["NCORES"]
    CPB = NCORES // B
    TOK = L // CPB              # 512 queries per core
    KT = L // P                 # 16 k-token chunks
    KTP = KT // 2               # 8 chunk pairs
    NJ = D // 256               # 4 D-contraction pairs
    NJF = DFF // 256            # 16 DFF-contraction pairs
    NM1 = DFF // P              # 32 mm1 output groups
    NTQ = TOK // P              # 4 query tiles
    HP = H // 2                 # 8 head pairs
    VW = H * 65 + DH    # vaug row width (65/head + pad; must stay EVEN)

    nc = bacc.Bacc(None, target_bir_lowering=False, debug=False)
    with tile.TileContext(nc) as tc, ExitStack() as top, \
            nc.allow_low_precision(reason="fp8 datapath, fp32 accumulate"):
        dram = top.enter_context(tc.tile_pool(name="dram", bufs=1, space="DRAM"))

        def din(name, shape, dtype=E4):
            return dram.tile(shape, dtype, kind="ExternalInput", name=name,
                             uniquify=False)

        xt2_d = din("xt2", [P, NJ, 2, L])
        xtq2_d = din("xtq2", [P, NJ, 2, TOK])
        wq2_d = din("wq2", [HP, P, NJ, 2, P])
        wk2_d = din("wk2", [HP, P, NJ, 2, P])     # pre-scaled by 1/8
        wv2_d = din("wv2", [P, NJ, 2, D])
        wo2_d = din("wo2", [DH, 2, HP, D])
        w1h_d = din("w1h", [NM1, P, NJ, 2, P])    # g1-folded, hi
        w1l_d = din("w1l", [NM1, P, NJ, 2, P], E5)
        w2h_d = din("w2h", [P, NJF, 2, D])
        w2l_d = din("w2l", [P, NJF, 2, D], E5)
        mt2_d = din("mt2", [DH, KT * 2 * TOK])    # binary mask (1 = masked)
        id2_d = din("id2", [DH, 2, P])            # -192 * packed identity
        xq_d = din("xq", [TOK, D], F16)           # x slice + bo + bv@wo
        bq_d = din("bq", [D], F32)
        bk_d = din("bk", [D], F32)                # pre-scaled by 1/8
        b1_d = din("b1", [DFF], F32)              # b1 + ln1_b @ w1
        g1_d = din("g1", [D], F16)
        bb2_d = din("bb2", [D], F16)              # ln1_b + b2
        g2_d = din("g2", [D], F16)
        be2_d = din("be2", [D], F16)
        out_d = dram.tile([TOK, D], F32, kind="ExternalOutput", name="out",
                          uniquify=False)

        def bcast_row(src_ap):
            return bass.AP(tensor=src_ap.tensor, offset=src_ap.offset,
                           ap=[[0, P]] + [list(a) for a in src_ap.ap])

        const = top.enter_context(tc.tile_pool(name="const", bufs=1))
        ident = const.tile([P, P], F32, name="ident")
        make_identity(nc, ident)
        id2_sb = const.tile([DH, 2, P], E4, name="id2_sb")
        nc.sync.dma_start(out=id2_sb, in_=id2_d[:])
        sh_t = const.tile([P, 1], F32, name="sh_t")
        nc.vector.memset(sh_t[:], -2.0)
        eps_t = const.tile([P, 1], F32, name="eps_t")
        nc.vector.memset(eps_t[:], LN_EPS)
        ones65 = const.tile([DH + 1, DH], F16, name="ones65")
        nc.vector.memset(ones65[:], 1.0)
        bq_sb = const.tile([P, HP], F32, name="bq_sb")
        nc.sync.dma_start(out=bq_sb, in_=bq_d[:].rearrange("(c p) -> p c", p=P))
        bk_sb = const.tile([P, HP], F32, name="bk_sb")
        nc.sync.dma_start(out=bk_sb, in_=bk_d[:].rearrange("(c p) -> p c", p=P))
        b1_sb = const.tile([P, NM1], F32, name="b1_sb")
        nc.sync.dma_start(out=b1_sb, in_=b1_d[:].rearrange("(c p) -> p c", p=P))
        invd_t = const.tile([P, 1], F32, name="invd_t")
        nc.vector.memset(invd_t[:], 1.0 / D)

        # cross-phase tiles
        xfer = top.enter_context(tc.tile_pool(name="xfer", bufs=1))
        h_t = [xfer.tile([P, D], F32, name=f"h{t}", tag=f"h{t}")
               for t in range(NTQ)]
        hTh = xfer.tile([P, NJ, 2, TOK], E4, name="hTh")
        hTl = xfer.tile([P, NJ, 2, TOK], E4, name="hTl")
        w2h_sb = xfer.tile([P, NJF, 2, D], E4, name="w2h_sb")
        w1h_s = {}
        w1l_s = {}
        for m in range(NM1):
            w1h_s[m] = xfer.tile([P, NJ, 2, P], E4, name="w1h_s",
                                 tag=f"w1h{m % 3}", bufs=1)
            w1l_s[m] = xfer.tile([P, NJ, 2, P], E5, name="w1l_s",
                                 tag=f"w1l{m % 3}", bufs=1)

        with ExitStack() as mid:
            pxo = mid.enter_context(tc.tile_pool(name="xo_sb", bufs=1))
            oT2 = pxo.tile([DH, HP, 2, TOK], E4, name="oT2")
            wo2_sb = pxo.tile([DH, 2, HP, D], E4, name="wo2_sb")
            xq_sb = [pxo.tile([P, D], F16, name=f"xq{t}", tag=f"xq{t}")
                     for t in range(NTQ)]
            g1_bc = pxo.tile([P, D], F16, name="g1_bc")
            bb2_bc = pxo.tile([P, D], F16, name="bb2_bc")

            # ---------------- attention ------------------------------------
            with ExitStack() as attn:
                pa = attn.enter_context(tc.tile_pool(name="attn_sb", bufs=1))
                pwk = attn.enter_context(tc.tile_pool(name="attn_wk", bufs=2))
                paw = attn.enter_context(tc.tile_pool(name="attn_w", bufs=1))

                mt2_s = pa.tile([DH, KT * 2 * TOK], E4, name="mt2_s")
                mt2_v = mt2_s[:].rearrange("c (k i q) -> c k i q", i=2, q=TOK)
                # vaug: per (ktp, i): 16 heads x 65 (V cols + ones) + pad
                vaug = pa.tile([P, KTP, 2, VW], E4, name="vaug")
                nc.vector.memset(
                    vaug[:, :, :, 0:H * 65].rearrange(
                        "p k i (h w) -> p k i h w", w=65)[:, :, :, :, 64:65],
                    1.0)
                nc.vector.memset(vaug[:, :, :, H * 65:VW], 0.0)

                xt2_s = paw.tile([P, NJ, 2, L], E4, name="xt2_s")
                xtq2_s = paw.tile([P, NJ, 2, TOK], E4, name="xtq2_s")

                # V first: its weights + x lead the DMA queue
                with tc.tile_pool(name="attn_v", bufs=1) as pav:
                    wv_s = pav.tile([P, NJ, 2, D], E4, name="wv_s")
                    nc.sync.dma_start(out=wv_s, in_=wv2_d[:])
                    for sl in range(4):
                        nc.sync.dma_start(
                            out=xt2_s[:, :, :, sl * 512:(sl + 1) * 512],
                            in_=xt2_d[:, :, :, sl * 512:(sl + 1) * 512])
                    nc.sync.dma_start(out=xtq2_s, in_=xtq2_d[:])

                    with tc.tile_pool(name="psV", bufs=1, space="PSUM") as psV:
                        for tc16 in range(KT):
                            vp = psV.tile([P, D], F32, name="vp", tag="vp",
                                          bufs=2)
                            for nch in range(4):
                                for j in range(NJ):
                                    nc.tensor.matmul(
                                        vp[:, nch * 256:(nch + 1) * 256],
                                        xt2_s[:, j, :,
                                              tc16 * P:(tc16 + 1) * P],
                                        wv_s[:, j, :,
                                             nch * 256:(nch + 1) * 256],
                                        start=(j == 0), stop=(j == NJ - 1),
                                        perf_mode=DR)
                            vdst = vaug[:, tc16 // 2, tc16 % 2,
                                        0:H * 65].rearrange(
                                "p (h w) -> p h w", w=65)[:, :, 0:DH]
                            vsrc = vp[:].rearrange("p (h w) -> p h w", w=DH)
                            if tc16 % 2 == 0:
                                nc.scalar.activation(vdst, vsrc, AF.Copy)
                            else:
                                nc.vector.tensor_copy(vdst, vsrc)

                # ---- streamed K/Q projection + scores/exp/AV pipeline ----
                ktq = [pa.tile([DH, 2, L], E4, name="ktq", tag=f"ktq{i % 2}",
                               bufs=1) for i in range(2)]
                qtq = [pa.tile([DH, 2, TOK], E4, name="qtq",
                               tag=f"qtq{i % 2}", bufs=1) for i in range(2)]
                es_t = [pa.tile([P, KTP, 2, TOK], E4, name="es",
                                tag=f"es{i}", bufs=1) for i in range(3)]
                wk_s = {}
                wq_s = {}
                for hp in range(HP):
                    wk_s[hp] = paw.tile([P, NJ, 2, P], E4, name="wk_s",
                                        tag=f"wk{hp % 3}", bufs=1)
                    wq_s[hp] = paw.tile([P, NJ, 2, P], E4, name="wq_s",
                                        tag=f"wq{hp % 3}", bufs=1)
                for hp in range(2):
                    nc.sync.dma_start(out=wk_s[hp], in_=wk2_d[hp])
                    nc.sync.dma_start(out=wq_s[hp], in_=wq2_d[hp])

                def kq_proj(hp, psB):
                    if hp + 2 < HP:
                        nc.sync.dma_start(out=wk_s[hp + 2], in_=wk2_d[hp + 2])
                        nc.sync.dma_start(out=wq_s[hp + 2], in_=wq2_d[hp + 2])
                    kt_t = ktq[hp % 2]
                    kf8 = pwk.tile([P, L], E4, name="kf8", tag="kf8")
                    for tg in range(4):
                        kp = psB.tile([P, 512], F32, name="kp", tag="kp",
                                      bufs=1)
                        for nq in range(2):
                            for j in range(NJ):
                                nc.tensor.matmul(
                                    kp[:, nq * 256:(nq + 1) * 256],
                                    wk_s[hp][:, j, :, :],
                                    xt2_s[:, j, :,
                                          tg * 512 + nq * 256:
                                          tg * 512 + (nq + 1) * 256],
                                    start=(j == 0), stop=(j == NJ - 1),
                                    perf_mode=DR)
                        nc.vector.tensor_scalar(
                            kf8[:, tg * 512:(tg + 1) * 512], kp,
                            bk_sb[:, hp:hp + 1], None, ALU.add)
                    for hh in range(2):
                        for i in range(2):
                            nc.sync.dma_start(
                                out=kt_t[32 * hh:32 * hh + 32, i, :],
                                in_=kf8[64 * hh + 32 * i:
                                        64 * hh + 32 * (i + 1), :])
                    qt_t = qtq[hp % 2]
                    qp = psB.tile([P, TOK], F32, name="qp", tag="kp", bufs=1)
                    for nq in range(2):
                        for j in range(NJ):
                            nc.tensor.matmul(
                                qp[:, nq * 256:(nq + 1) * 256],
                                wq_s[hp][:, j, :, :],
                                xtq2_s[:, j, :, nq * 256:(nq + 1) * 256],
                                start=(j == 0), stop=(j == NJ - 1),
                                perf_mode=DR)
                    qf8 = pwk.tile([P, TOK], E4, name="qf8", tag="qf8")
                    nc.vector.tensor_scalar(qf8, qp, bq_sb[:, hp:hp + 1],
                                            None, ALU.add)
                    for hh in range(2):
                        for i in range(2):
                            nc.sync.dma_start(
                                out=qt_t[32 * hh:32 * hh + 32, i, :],
                                in_=qf8[64 * hh + 32 * i:
                                        64 * hh + 32 * (i + 1), :])

                def scores_exp(h, psB):
                    hp, hh = h // 2, h % 2
                    base = 32 * hh
                    kt_t, qt_t = ktq[hp % 2], qtq[hp % 2]
                    es = es_t[h % 3]
                    for ktp in range(KTP):
                        sp = psB.tile([P, 2, TOK], F32, name="sp",
                                      tag="sp", bufs=2)
                        for i2 in range(2):
                            kt = 2 * ktp + i2
                            for qh in range(2):
                                qs = slice(qh * 256, (qh + 1) * 256)
                                nc.tensor.matmul(
                                    sp[:, i2, qs],
                                    kt_t[base:base + 32, :,
                                         kt * P:(kt + 1) * P],
                                    qt_t[base:base + 32, :, qs],
                                    start=True, stop=False, perf_mode=DR)
                                nc.tensor.matmul(
                                    sp[:, i2, qs], id2_sb[:],
                                    mt2_v[:, kt, :, qs],
                                    start=False, stop=True, perf_mode=DR)
                        nc.scalar.activation(es[:, ktp, :, :], sp, AF.Exp,
                                             bias=sh_t)

                def av_norm(h, psB):
                    hp, hh = h // 2, h % 2
                    es = es_t[h % 3]
                    otp = psB.tile([P, TOK], F32, name="otp", tag="otp",
                                   bufs=2)
                    # qh-outer: each half-bank group runs start->stop without
                    # another group's start in between
                    for qh in range(2):
                        qs = slice(qh * 256, (qh + 1) * 256)
                        for ktp in range(KTP):
                            nc.tensor.matmul(
                                otp[:, qs],
                                vaug[:, ktp, :, 65 * h:65 * h + P],
                                es[:, ktp, :, qs],
                                start=(ktp == 0), stop=(ktp == KTP - 1),
                                perf_mode=DR)
                    rt = pwk.tile([DH + 1, TOK], F16, name="rt", tag="rt",
                                  bufs=1)
                    nc.vector.reciprocal(rt[DH:DH + 1, :], otp[DH:DH + 1, :])
                    rb = psB.tile([DH, TOK], F32, name="rb", tag="rb", bufs=1)
                    nc.tensor.matmul(rb, ones65[DH:DH + 1, :],
                                     rt[DH:DH + 1, :], start=True, stop=True)
                    rbs = pwk.tile([DH, TOK], F16, name="rbs", tag="rbs",
                                   bufs=1)
                    nc.vector.tensor_copy(rbs, rb)
                    nc.vector.tensor_tensor(oT2[:, hp, hh, :],
                                            otp[0:DH, :], rbs, ALU.mult)

                nc.sync.dma_start(out=mt2_s, in_=mt2_d[:])
                with tc.tile_pool(name="psB", bufs=1, space="PSUM") as psB:
                    kq_proj(0, psB)
                    for hp in range(HP):
                        if hp + 1 < HP:
                            kq_proj(hp + 1, psB)
                        if hp == 3:
                            # O-proj phase prefetches, in the DMA-idle window
                            nc.sync.dma_start(out=wo2_sb, in_=wo2_d[:])
                            xq_r = xq_d[:].rearrange("(t p) d -> t p d", p=P)
                            for t in range(NTQ):
                                nc.sync.dma_start(out=xq_sb[t], in_=xq_r[t])
                            nc.sync.dma_start(out=g1_bc,
                                              in_=bcast_row(g1_d[:]))
                            nc.sync.dma_start(out=bb2_bc,
                                              in_=bcast_row(bb2_d[:]))
                            for m in range(2):
                                nc.sync.dma_start(out=w1h_s[m], in_=w1h_d[m])
                                nc.sync.dma_start(out=w1l_s[m], in_=w1l_d[m])
                        if 4 <= hp <= 7:
                            jfs = 4 * (hp - 4)
                            nc.sync.dma_start(
                                out=w2h_sb[:, jfs:jfs + 4, :, :],
                                in_=w2h_d[:, jfs:jfs + 4, :, :])
                        scores_exp(2 * hp, psB)
                        if hp > 0:
                            av_norm(2 * hp - 1, psB)
                        scores_exp(2 * hp + 1, psB)
                        av_norm(2 * hp, psB)
                    av_norm(H - 1, psB)

            # FFN pools open early so w2l/w1 can stream during phase C
            dph = ExitStack()
            pdw = dph.enter_context(tc.tile_pool(name="d_w", bufs=1))
            pdwk = dph.enter_context(tc.tile_pool(name="d_wk", bufs=3))
            g2_bc = pdw.tile([P, D], F16, name="g2_bc")
            be2_bc = pdw.tile([P, D], F16, name="be2_bc")
            nc.sync.dma_start(out=g2_bc, in_=bcast_row(g2_d[:]))
            nc.sync.dma_start(out=be2_bc, in_=bcast_row(be2_d[:]))
            f1h = pdw.tile([P, NJF, 2, TOK], E4, name="f1h")
            f1l = pdw.tile([P, NJF, 2, TOK], E4, name="f1l")
            w2l_sb = pdw.tile([P, NJF, 2, D], E5, name="w2l_sb")

            # ------------- O-projection + LN1 + transpose ------------------
            def ln_normalize(x_tile, wk):
                st = wk.tile([P, 2, 6], F32, name="lnst", tag="lnst")
                xv = x_tile.rearrange("p (s f) -> p s f", f=512)
                for sg in range(2):
                    nc.vector.bn_stats(out=st[:, sg, :], in_=xv[:, sg, :])
                mv = wk.tile([P, 2], F32, name="lnmv", tag="lnmv")
                nc.vector.bn_aggr(out=mv, in_=st)
                sq = wk.tile([P, 1], F32, name="lnsq", tag="lnsq")
                nc.scalar.activation(sq, mv[:, 1:2], AF.Sqrt, bias=eps_t)
                nc.vector.reciprocal(sq, sq)
                nc.gpsimd.tensor_scalar(x_tile, x_tile, mv[:, 0:1], sq,
                                        ALU.subtract, ALU.mult)

            with ExitStack() as cph:
                pcwk = cph.enter_context(tc.tile_pool(name="c_wk", bufs=3))
                pcp = cph.enter_context(tc.tile_pool(name="c_ps", bufs=1,
                                                     space="PSUM"))
                for t in range(NTQ):
                    op = pcp.tile([P, D], F32, name="op", tag="op", bufs=2)
                    for nch in range(4):
                        for hp in range(HP):
                            nc.tensor.matmul(
                                op[:, nch * 256:(nch + 1) * 256],
                                oT2[:, hp, :, t * P:(t + 1) * P],
                                wo2_sb[:, :, hp, nch * 256:(nch + 1) * 256],
                                start=(hp == 0), stop=(hp == HP - 1),
                                perf_mode=DR)
                    s1 = pcwk.tile([P, 1], F32, name="s1", tag="s1")
                    nc.vector.scalar_tensor_tensor(
                        h_t[t], op, 0.0, xq_sb[t], ALU.bypass, ALU.add,
                        accum_out=s1)
                    scr = pcwk.tile([P, D], F16, name="scr", tag="scr")
                    s2 = pcwk.tile([P, 1], F32, name="s2", tag="s2")
                    nc.scalar.activation(scr, h_t[t], AF.Square, accum_out=s2)
                    mean = pcwk.tile([P, 1], F32, name="mean", tag="mean")
                    nc.vector.tensor_scalar(mean, s1, invd_t, None, ALU.mult)
                    var = pcwk.tile([P, 1], F32, name="var", tag="var")
                    nc.vector.scalar_tensor_tensor(var, mean, 0.0, mean,
                                                   ALU.bypass, ALU.mult)
                    nc.vector.scalar_tensor_tensor(var, s2, invd_t, var,
                                                   ALU.mult, ALU.subtract)
                    sq = pcwk.tile([P, 1], F32, name="lnsq", tag="lnsq")
                    nc.scalar.activation(sq, var, AF.Sqrt, bias=eps_t)
                    nc.vector.reciprocal(sq, sq)
                    nc.gpsimd.tensor_scalar(h_t[t], h_t[t], mean, sq,
                                            ALU.subtract, ALU.mult)
                    for half in range(2):
                        tp = pcp.tile([P, 512], F32, name="tp", tag="tp",
                                      bufs=2)
                        for c4 in range(4):
                            c = half * 4 + c4
                            nc.tensor.transpose(
                                tp[:, c4 * P:(c4 + 1) * P],
                                h_t[t][:, c * P:(c + 1) * P], ident)
                        hs = slice(2 * half, 2 * half + 2)
                        ts_ = slice(t * P, (t + 1) * P)
                        tpr = tp[:].rearrange("p (j i c) -> p j i c", j=2, i=2)
                        nc.scalar.activation(hTh[:, hs, :, ts_], tpr, AF.Copy)
                        nc.vector.tensor_tensor(hTl[:, hs, :, ts_], tpr,
                                                hTh[:, hs, :, ts_],
                                                ALU.subtract)
                    # residual term of the final sum: h <- h*g1 + (ln1_b+b2)
                    nc.gpsimd.tensor_tensor(h_t[t], h_t[t], g1_bc, ALU.mult)
                    nc.gpsimd.tensor_tensor(h_t[t], h_t[t], bb2_bc, ALU.add)

        # ---------------- FFN ---------------------------------------------
        with ExitStack() as dph:
            pdw = dph.enter_context(tc.tile_pool(name="d_w", bufs=1))
            pdwk = dph.enter_context(tc.tile_pool(name="d_wk", bufs=3))
            g2_bc = pdw.tile([P, D], F16, name="g2_bc")
            be2_bc = pdw.tile([P, D], F16, name="be2_bc")
            nc.sync.dma_start(out=g2_bc, in_=bcast_row(g2_d[:]))
            nc.sync.dma_start(out=be2_bc, in_=bcast_row(be2_d[:]))
            f1h = pdw.tile([P, NJF, 2, TOK], E4, name="f1h")
            f1l = pdw.tile([P, NJF, 2, TOK], E4, name="f1l")
            w2l_sb = pdw.tile([P, NJF, 2, D], E5, name="w2l_sb")

            with tc.tile_pool(name="d_ps1", bufs=1, space="PSUM") as pd1:
                for m in range(NM1):
                    if m + 2 < NM1:
                        nc.sync.dma_start(out=w1h_s[m + 2], in_=w1h_d[m + 2])
                        nc.sync.dma_start(out=w1l_s[m + 2], in_=w1l_d[m + 2])
                    jf, im = m // 2, m % 2
                    fp = pd1.tile([P, TOK], F32, name="fp", tag="fp", bufs=2)
                    for th in range(2):
                        ts_ = slice(th * 256, (th + 1) * 256)
                        fps = fp[:, ts_]
                        for j in range(NJ):
                            nc.tensor.matmul(fps, w1h_s[m][:, j, :, :],
                                             hTh[:, j, :, ts_],
                                             start=(j == 0), stop=False,
                                             perf_mode=DR)
                            nc.tensor.matmul(fps, w1h_s[m][:, j, :, :],
                                             hTl[:, j, :, ts_],
                                             start=False, stop=False,
                                             perf_mode=DR)
                            nc.tensor.matmul(fps, w1l_s[m][:, j, :, :],
                                             hTh[:, j, :, ts_],
                                             start=False, stop=(j == NJ - 1),
                                             perf_mode=DR)
                        f1f = pdwk.tile([P, 256], F16, name="f1f", tag="f1f")
                        nc.scalar.activation(f1f, fps, AF.Relu,
                                             bias=b1_sb[:, m:m + 1])
                        nc.gpsimd.tensor_copy(f1h[:, jf, im, ts_], f1f)
                        nc.gpsimd.tensor_tensor(f1l[:, jf, im, ts_], f1f,
                                                f1h[:, jf, im, ts_],
                                                ALU.subtract)

            # mm2: w2 fully resident; per-(t, nch) groups run start->stop
            # contiguously so shared PSUM banks see no interleaved starts
            with tc.tile_pool(name="d_ps2", bufs=1, space="PSUM") as pd2:
                for t in range(NTQ):
                    ts_ = slice(t * P, (t + 1) * P)
                    g2p_t = pd2.tile([P, D], F32, name="g2p", tag="g2p",
                                     bufs=2)
                    for nch in range(4):
                        ns = slice(nch * 256, (nch + 1) * 256)
                        for jf in range(NJF):
                            nc.tensor.matmul(g2p_t[:, ns],
                                             f1h[:, jf, :, ts_],
                                             w2h_sb[:, jf, :, ns],
                                             start=(jf == 0), stop=False,
                                             perf_mode=DR)
                            nc.tensor.matmul(g2p_t[:, ns],
                                             f1l[:, jf, :, ts_],
                                             w2h_sb[:, jf, :, ns],
                                             start=False, stop=False,
                                             perf_mode=DR)
                            nc.tensor.matmul(g2p_t[:, ns],
                                             f1h[:, jf, :, ts_],
                                             w2l_sb[:, jf, :, ns],
                                             start=False,
                                             stop=(jf == NJF - 1),
                                             perf_mode=DR)
                    f2 = pdwk.tile([P, D], F32, name="f2", tag="f2", bufs=2)
                    halves = [slice(0, 512), slice(512, D)]
                    for hs in halves:
                        nc.vector.tensor_tensor(f2[:, hs], h_t[t][:, hs],
                                                g2p_t[:, hs], ALU.add)
                    st = pdwk.tile([P, 2, 6], F32, name="lnst", tag="lnst")
                    for sg in range(2):
                        nc.vector.bn_stats(
                            out=st[:, sg, :],
                            in_=f2[:, sg * 512:(sg + 1) * 512])
                    mv = pdwk.tile([P, 2], F32, name="lnmv", tag="lnmv")
                    nc.vector.bn_aggr(out=mv, in_=st)
                    sq = pdwk.tile([P, 1], F32, name="lnsq", tag="lnsq")
                    nc.scalar.activation(sq, mv[:, 1:2], AF.Sqrt, bias=eps_t)
                    nc.vector.reciprocal(sq, sq)
                    for hs in halves:
                        nc.gpsimd.tensor_scalar(f2[:, hs], f2[:, hs],
                                                mv[:, 0:1], sq,
                                                ALU.subtract, ALU.mult)
                        nc.gpsimd.tensor_tensor(f2[:, hs], f2[:, hs],
                                                g2_bc[:, hs], ALU.mult)
                        nc.gpsimd.tensor_tensor(f2[:, hs], f2[:, hs],
                                                be2_bc[:, hs], ALU.add)
                        nc.sync.dma_start(out=out_d[t * P:(t + 1) * P, hs],
                                          in_=f2[:, hs])

    nc.compile()
    return nc


def _pack_dr(w):
    """[D, N] -> [128, D//256, 2, N] (contraction chunk-pairs)."""
    Dd, N = w.shape
    return np.ascontiguousarray(
        w.reshape(Dd // 256, 2, P, N).transpose(2, 0, 1, 3))


def make_in_maps(cfg, inp):
    B, L, D, H, DFF = cfg["B"], cfg["L"], cfg["D"], cfg["H"], cfg["DFF"]
    NCORES = cfg["NCORES"]
    CPB = NCORES // B
    TOK = L // CPB
    KT = L // P
    NM1 = DFF // P
    HPn = H // 2
    f32 = np.float32
    x = np.asarray(inp["x"], f32)
    mask = np.asarray(inp["mask"], bool)
    w = {k: np.asarray(inp[k], f32) for k in
         ("wq", "bq", "wk", "bk", "wv", "bv", "wo", "bo", "w1", "b1",
          "w2", "b2", "ln1_g", "ln1_b", "ln2_g", "ln2_b")}
    bo2 = w["bo"] + w["bv"] @ w["wo"]
    w1s = w["ln1_g"][:, None] * w["w1"]
    b1s = w["b1"] + w["ln1_b"] @ w["w1"]
    bb2 = w["ln1_b"] + w["b2"]

    def hilo(a):
        hi = a.astype(E4NP)
        lo = (a - hi.astype(f32)).astype(E5NP)
        return hi, lo

    w1hf, w1lf = hilo(w1s)
    w2hf, w2lf = hilo(w["w2"])
    w1h = np.stack([_pack_dr(w1hf.astype(f32)[:, m * P:(m + 1) * P])
                    for m in range(NM1)]).astype(E4NP)
    w1l = np.stack([_pack_dr(w1lf.astype(f32)[:, m * P:(m + 1) * P])
                    for m in range(NM1)]).astype(E5NP)
    w2h = _pack_dr(w2hf.astype(f32)).astype(E4NP)
    w2l = _pack_dr(w2lf.astype(f32)).astype(E5NP)
    wo2 = np.ascontiguousarray(
        w["wo"].reshape(HPn, 2, DH, D).transpose(2, 1, 0, 3)).astype(E4NP)
    # wk/wq: per-head-pair slabs [HP, 128, NJ, 2, 128]
    wk2 = np.stack([_pack_dr(w["wk"][:, hp * P:(hp + 1) * P] * 0.125)
                    for hp in range(HPn)]).astype(E4NP)
    wq2 = np.stack([_pack_dr(w["wq"][:, hp * P:(hp + 1) * P])
                    for hp in range(HPn)]).astype(E4NP)
    id2 = np.zeros((DH, 2, P), f32)
    for i in range(2):
        for c in range(DH):
            id2[c, i, c + DH * i] = -192.0
    shared = dict(
        wq2=wq2, wk2=wk2,
        wv2=_pack_dr(w["wv"]).astype(E4NP),
        wo2=wo2, w1h=w1h, w1l=w1l, w2h=w2h, w2l=w2l,
        id2=id2.astype(E4NP),
        bq=w["bq"], bk=w["bk"] * f32(0.125), b1=b1s,
        g1=w["ln1_g"].astype(np.float16), bb2=bb2.astype(np.float16),
        g2=w["ln2_g"].astype(np.float16), be2=w["ln2_b"].astype(np.float16))
    shared = {k: np.ascontiguousarray(v) for k, v in shared.items()}
    in_maps = []
    for c in range(NCORES):
        b, q0 = c // CPB, (c % CPB) * TOK
        xb = x[b]
        m = dict(shared)
        m["xt2"] = _pack_dr(xb.T).astype(E4NP)
        m["xtq2"] = _pack_dr(np.ascontiguousarray(xb[q0:q0 + TOK]).T
                             ).astype(E4NP)
        m["xq"] = np.ascontiguousarray(
            (xb[q0:q0 + TOK] + bo2).astype(np.float16))
        # mt2[c2, kt, i, q] = mask[b, q0+q, kt*128 + i*64 + c2]
        mt = mask[b, q0:q0 + TOK, :].T.astype(f32)  # [L, TOK]
        m["mt2"] = np.ascontiguousarray(
            mt.reshape(KT, 2, DH, TOK).transpose(2, 0, 1, 3)
            .reshape(DH, KT * 2 * TOK)).astype(E4NP)
        in_maps.append(m)
    return in_maps


_NC_CACHE = {}
TRACE = False
LAST_RESULTS = None


def _get_nc(key, cfg):
    if key not in _NC_CACHE:
        _NC_CACHE[key] = build_bass(cfg)
    return _NC_CACHE[key]


def kernel(**inputs):
    global LAST_RESULTS
    from concourse.bass_utils import run_bass_kernel_spmd

    cfg = FULL_CFG
    B, L, D = cfg["B"], cfg["L"], cfg["D"]
    NCORES = cfg["NCORES"]
    CPB = NCORES // B
    TOK = L // CPB
    nc = _get_nc("full", cfg)
    in_maps = make_in_maps(cfg, inputs)
    res = run_bass_kernel_spmd(nc, in_maps, core_ids=list(range(NCORES)),
                               trace=TRACE)
    LAST_RESULTS = res
    out = np.empty((B, L, D), np.float32)
    for c in range(NCORES):
        b, q0 = c // CPB, (c % CPB) * TOK
        out[b, q0:q0 + TOK] = res.results[c]["out"]
    return out


# revision 3
# speedup vs baseline: 1.0232x; 1.0232x over previous
"""Trainium2 Bass kernel for a dense transformer encoder block — fp8 DoubleRow.

Sharding (8 cores): sequence-parallel. Core c handles batch b = c//4 and the
512-token query slice q0 = (c%4)*512; K/V are computed for the full batch on
each core.

Datapath: fp8 e4m3 with DoubleRow matmuls (0.5 cyc/row, 2x contraction packed
into the free dim -> 4x the fp16 MAC rate at K=128). The LxL mask is applied
by an extra DoubleRow matmul (-192*I stationary x binary mask moving)
accumulated straight into the scores PSUM; exp(s-2) then goes PSUM->fp8 on the
ACT engine (the -2 shift keeps e4m3 from overflowing and cancels in the
softmax normalization). Scores contract dh=64 as [32,2] (K/Q folded via
partition-shifted DMAs); AV contracts k-token chunk PAIRS as [128,2] with V
laid out 65-wide per head, read through a 128-wide overlapping AP. The FFN
runs 3-term hi/lo fp8 (ah@bh + al@bh + ah@bl, lo-weights in e5m2).

Schedule: V projection runs first under the initial DMA shadow (V copies on
DVE); then K/Q projection + fold stream one head-pair ahead of a
scores->exp->AV software pipeline (AV lags exp by one head), so the ACT
engine's exp stream - the critical resource - starts early and never waits.
PSUM accumulation groups never interleave with another group's start in the
same bank (hardware lazily zeroes the whole 2KB bank on start_tensor_calc).
"""

import sys
from contextlib import ExitStack

import numpy as np

for _p in ("/opt/trn_rl_repo", "/opt/pypackages"):
    if _p not in sys.path:
        sys.path.append(_p)

import ml_dtypes  # noqa: E402
import concourse.bass as bass  # noqa: E402
import concourse.tile as tile  # noqa: E402
from concourse import bacc, mybir  # noqa: E402
from concourse.masks import make_identity  # noqa: E402

F32 = mybir.dt.float32
F16 = mybir.dt.float16
E4 = mybir.dt.float8e4
E5 = mybir.dt.float8e5
E4NP = ml_dtypes.float8_e4m3
E5NP = ml_dtypes.float8_e5m2
AF = mybir.ActivationFunctionType
ALU = mybir.AluOpType
PM = mybir.MatmulPerfMode
DR = PM.DoubleRow

P = 128
DH = 64
LN_EPS = 1e-5

FULL_CFG = dict(B=2, L=2048, D=1024, H=16, DFF=4096, NCORES=8)


def build_bass(cfg):
    B, L, D, H, DFF = cfg["B"], cfg["L"], cfg["D"], cfg["H"], cfg["DFF"]
    NCORES = cfg["NCORES"]
    CPB = NCORES // B
    TOK = L // CPB              # 512 queries per core
    KT = L // P                 # 16 k-token chunks
    KTP = KT // 2               # 8 chunk pairs
    NJ = D // 256               # 4 D-contraction pairs
    NJF = DFF // 256            # 16 DFF-contraction pairs
    NM1 = DFF // P              # 32 mm1 output groups
    NTQ = TOK // P              # 4 query tiles
    HP = H // 2                 # 8 head pairs
    VW = H * 65 + DH    # vaug row width (65/head + pad; must stay EVEN)

    nc = bacc.Bacc(None, target_bir_lowering=False, debug=False)
    with tile.TileContext(nc) as tc, ExitStack() as top, \
            nc.allow_low_precision(reason="fp8 datapath, fp32 accumulate"):
        dram = top.enter_context(tc.tile_pool(name="dram", bufs=1, space="DRAM"))

        def din(name, shape, dtype=E4):
            return dram.tile(shape, dtype, kind="ExternalInput", name=name,
                             uniquify=False)

        xt2_d = din("xt2", [P, NJ, 2, L])
        xtq2_d = din("xtq2", [P, NJ, 2, TOK])
        wq2_d = din("wq2", [HP, P, NJ, 2, P])
        wk2_d = din("wk2", [HP, P, NJ, 2, P])     # pre-scaled by 1/8
        wv2_d = din("wv2", [P, NJ, 2, D])
        wo2_d = din("wo2", [DH, 2, HP, D])
        w1h_d = din("w1h", [NM1, P, NJ, 2, P])    # g1-folded, hi
        w1l_d = din("w1l", [NM1, P, NJ, 2, P], E5)
        w2h_d = din("w2h", [P, NJF, 2, D])
        w2l_d = din("w2l", [P, NJF, 2, D], E5)
        mt2_d = din("mt2", [DH, KT * 2 * TOK])    # binary mask (1 = masked)
        id2_d = din("id2", [DH, 2, P])            # -192 * packed identity
        xq_d = din("xq", [TOK, D], F16)           # x slice + bo + bv@wo
        bq_d = din("bq", [D], F32)
        bk_d = din("bk", [D], F32)                # pre-scaled by 1/8
        b1_d = din("b1", [DFF], F32)              # b1 + ln1_b @ w1
        g1_d = din("g1", [D], F16)
        bb2_d = din("bb2", [D], F16)              # ln1_b + b2
        g2_d = din("g2", [D], F16)
        be2_d = din("be2", [D], F16)
        out_d = dram.tile([TOK, D], F32, kind="ExternalOutput", name="out",
                          uniquify=False)

        def bcast_row(src_ap):
            return bass.AP(tensor=src_ap.tensor, offset=src_ap.offset,
                           ap=[[0, P]] + [list(a) for a in src_ap.ap])

        const = top.enter_context(tc.tile_pool(name="const", bufs=1))
        ident = const.tile([P, P], F32, name="ident")
        make_identity(nc, ident)
        id2_sb = const.tile([DH, 2, P], E4, name="id2_sb")
        nc.sync.dma_start(out=id2_sb, in_=id2_d[:])
        sh_t = const.tile([P, 1], F32, name="sh_t")
        nc.vector.memset(sh_t[:], -2.0)
        eps_t = const.tile([P, 1], F32, name="eps_t")
        nc.vector.memset(eps_t[:], LN_EPS)
        ones65 = const.tile([DH + 1, DH], F16, name="ones65")
        nc.vector.memset(ones65[:], 1.0)
        bq_sb = const.tile([P, HP], F32, name="bq_sb")
        nc.sync.dma_start(out=bq_sb, in_=bq_d[:].rearrange("(c p) -> p c", p=P))
        bk_sb = const.tile([P, HP], F32, name="bk_sb")
        nc.sync.dma_start(out=bk_sb, in_=bk_d[:].rearrange("(c p) -> p c", p=P))
        b1_sb = const.tile([P, NM1], F32, name="b1_sb")
        nc.sync.dma_start(out=b1_sb, in_=b1_d[:].rearrange("(c p) -> p c", p=P))
        invd_t = const.tile([P, 1], F32, name="invd_t")
        nc.vector.memset(invd_t[:], 1.0 / D)

        # cross-phase tiles
        xfer = top.enter_context(tc.tile_pool(name="xfer", bufs=1))
        h_t = [xfer.tile([P, D], F32, name=f"h{t}", tag=f"h{t}")
               for t in range(NTQ)]
        hTh = xfer.tile([P, NJ, 2, TOK], E4, name="hTh")
        hTl = xfer.tile([P, NJ, 2, TOK], E4, name="hTl")
        w2h_sb = xfer.tile([P, NJF, 2, D], E4, name="w2h_sb")
        w1h_s = {}
        w1l_s = {}
        for m in range(NM1):
            w1h_s[m] = xfer.tile([P, NJ, 2, P], E4, name="w1h_s",
                                 tag=f"w1h{m % 3}", bufs=1)
            w1l_s[m] = xfer.tile([P, NJ, 2, P], E5, name="w1l_s",
                                 tag=f"w1l{m % 3}", bufs=1)

        with ExitStack() as mid:
            pxo = mid.enter_context(tc.tile_pool(name="xo_sb", bufs=1))
            oT2 = pxo.tile([DH, HP, 2, TOK], E4, name="oT2")
            wo2_sb = pxo.tile([DH, 2, HP, D], E4, name="wo2_sb")
            xq_sb = [pxo.tile([P, D], F16, name=f"xq{t}", tag=f"xq{t}")
                     for t in range(NTQ)]
            g1_bc = pxo.tile([P, D], F16, name="g1_bc")
            bb2_bc = pxo.tile([P, D], F16, name="bb2_bc")

            # ---------------- attention ------------------------------------
            with ExitStack() as attn:
                pa = attn.enter_context(tc.tile_pool(name="attn_sb", bufs=1))
                pwk = attn.enter_context(tc.tile_pool(name="attn_wk", bufs=2))
                paw = attn.enter_context(tc.tile_pool(name="attn_w", bufs=1))

                mt2_s = pa.tile([DH, KT * 2 * TOK], E4, name="mt2_s")
                mt2_v = mt2_s[:].rearrange("c (k i q) -> c k i q", i=2, q=TOK)
                # vaug: per (ktp, i): 16 heads x 65 (V cols + ones) + pad
                vaug = pa.tile([P, KTP, 2, VW], E4, name="vaug")
                nc.vector.memset(
                    vaug[:, :, :, 0:H * 65].rearrange(
                        "p k i (h w) -> p k i h w", w=65)[:, :, :, :, 64:65],
                    1.0)
                nc.vector.memset(vaug[:, :, :, H * 65:VW], 0.0)

                xt2_s = paw.tile([P, NJ, 2, L], E4, name="xt2_s")
                xtq2_s = paw.tile([P, NJ, 2, TOK], E4, name="xtq2_s")

                # V first: its weights + x lead the DMA queue
                with tc.tile_pool(name="attn_v", bufs=1) as pav:
                    wv_s = pav.tile([P, NJ, 2, D], E4, name="wv_s")
                    nc.sync.dma_start(out=wv_s, in_=wv2_d[:])
                    for sl in range(4):
                        nc.sync.dma_start(
                            out=xt2_s[:, :, :, sl * 512:(sl + 1) * 512],
                            in_=xt2_d[:, :, :, sl * 512:(sl + 1) * 512])
                    nc.sync.dma_start(out=xtq2_s, in_=xtq2_d[:])

                    with tc.tile_pool(name="psV", bufs=1, space="PSUM") as psV:
                        for tc16 in range(KT):
                            vp = psV.tile([P, D], F32, name="vp", tag="vp",
                                          bufs=2)
                            for nch in range(4):
                                for j in range(NJ):
                                    nc.tensor.matmul(
                                        vp[:, nch * 256:(nch + 1) * 256],
                                        xt2_s[:, j, :,
                                              tc16 * P:(tc16 + 1) * P],
                                        wv_s[:, j, :,
                                             nch * 256:(nch + 1) * 256],
                                        start=(j == 0), stop=(j == NJ - 1),
                                        perf_mode=DR)
                            vdst = vaug[:, tc16 // 2, tc16 % 2,
                                        0:H * 65].rearrange(
                                "p (h w) -> p h w", w=65)[:, :, 0:DH]
                            vsrc = vp[:].rearrange("p (h w) -> p h w", w=DH)
                            if tc16 % 2 == 0:
                                nc.scalar.activation(vdst, vsrc, AF.Copy)
                            else:
                                nc.vector.tensor_copy(vdst, vsrc)

                # ---- streamed K/Q projection + scores/exp/AV pipeline ----
                ktq = [pa.tile([DH, 2, L], E4, name="ktq", tag=f"ktq{i % 2}",
                               bufs=1) for i in range(2)]
                qtq = [pa.tile([DH, 2, TOK], E4, name="qtq",
                               tag=f"qtq{i % 2}", bufs=1) for i in range(2)]
                es_t = [pa.tile([P, KTP, 2, TOK], E4, name="es",
                                tag=f"es{i}", bufs=1) for i in range(3)]
                wk_s = {}
                wq_s = {}
                for hp in range(HP):
                    wk_s[hp] = paw.tile([P, NJ, 2, P], E4, name="wk_s",
                                        tag=f"wk{hp % 3}", bufs=1)
                    wq_s[hp] = paw.tile([P, NJ, 2, P], E4, name="wq_s",
                                        tag=f"wq{hp % 3}", bufs=1)
                for hp in range(2):
                    nc.sync.dma_start(out=wk_s[hp], in_=wk2_d[hp])
                    nc.sync.dma_start(out=wq_s[hp], in_=wq2_d[hp])

                def kq_proj(hp, psB):
                    if hp + 2 < HP:
                        nc.sync.dma_start(out=wk_s[hp + 2], in_=wk2_d[hp + 2])
                        nc.sync.dma_start(out=wq_s[hp + 2], in_=wq2_d[hp + 2])
                    kt_t = ktq[hp % 2]
                    kf8 = pwk.tile([P, L], E4, name="kf8", tag="kf8")
                    for tg in range(4):
                        kp = psB.tile([P, 512], F32, name="kp", tag="kp",
                                      bufs=1)
                        for nq in range(2):
                            for j in range(NJ):
                                nc.tensor.matmul(
                                    kp[:, nq * 256:(nq + 1) * 256],
                                    wk_s[hp][:, j, :, :],
                                    xt2_s[:, j, :,
                                          tg * 512 + nq * 256:
                                          tg * 512 + (nq + 1) * 256],
                                    start=(j == 0), stop=(j == NJ - 1),
                                    perf_mode=DR)
                        nc.vector.tensor_scalar(
                            kf8[:, tg * 512:(tg + 1) * 512], kp,
                            bk_sb[:, hp:hp + 1], None, ALU.add)
                    for hh in range(2):
                        for i in range(2):
                            nc.sync.dma_start(
                                out=kt_t[32 * hh:32 * hh + 32, i, :],
                                in_=kf8[64 * hh + 32 * i:
                                        64 * hh + 32 * (i + 1), :])
                    qt_t = qtq[hp % 2]
                    qp = psB.tile([P, TOK], F32, name="qp", tag="kp", bufs=1)
                    for nq in range(2):
                        for j in range(NJ):
                            nc.tensor.matmul(
                                qp[:, nq * 256:(nq + 1) * 256],
                                wq_s[hp][:, j, :, :],
                                xtq2_s[:, j, :, nq * 256:(nq + 1) * 256],
                                start=(j == 0), stop=(j == NJ - 1),
                                perf_mode=DR)
                    qf8 = pwk.tile([P, TOK], E4, name="qf8", tag="qf8")
                    nc.vector.tensor_scalar(qf8, qp, bq_sb[:, hp:hp + 1],
                                            None, ALU.add)
                    for hh in range(2):
                        for i in range(2):
                            nc.sync.dma_start(
                                out=qt_t[32 * hh:32 * hh + 32, i, :],
                                in_=qf8[64 * hh + 32 * i:
                                        64 * hh + 32 * (i + 1), :])

                def scores_exp(h, psB):
                    hp, hh = h // 2, h % 2
                    base = 32 * hh
                    kt_t, qt_t = ktq[hp % 2], qtq[hp % 2]
                    es = es_t[h % 3]
                    for ktp in range(KTP):
                        sp = psB.tile([P, 2, TOK], F32, name="sp",
                                      tag="sp", bufs=2)
                        for i2 in range(2):
                            kt = 2 * ktp + i2
                            for qh in range(2):
                                qs = slice(qh * 256, (qh + 1) * 256)
                                nc.tensor.matmul(
                                    sp[:, i2, qs],
                                    kt_t[base:base + 32, :,
                                         kt * P:(kt + 1) * P],
                                    qt_t[base:base + 32, :, qs],
                                    start=True, stop=False, perf_mode=DR)
                                nc.tensor.matmul(
                                    sp[:, i2, qs], id2_sb[:],
                                    mt2_v[:, kt, :, qs],
                                    start=False, stop=True, perf_mode=DR)
                        nc.scalar.activation(es[:, ktp, :, :], sp, AF.Exp,
                                             bias=sh_t)

                def av_norm(h, psB):
                    hp, hh = h // 2, h % 2
                    es = es_t[h % 3]
                    otp = psB.tile([P, TOK], F32, name="otp", tag="otp",
                                   bufs=2)
                    # qh-outer: each half-bank group runs start->stop without
                    # another group's start in between
                    for qh in range(2):
                        qs = slice(qh * 256, (qh + 1) * 256)
                        for ktp in range(KTP):
                            nc.tensor.matmul(
                                otp[:, qs],
                                vaug[:, ktp, :, 65 * h:65 * h + P],
                                es[:, ktp, :, qs],
                                start=(ktp == 0), stop=(ktp == KTP - 1),
                                perf_mode=DR)
                    rt = pwk.tile([DH + 1, TOK], F16, name="rt", tag="rt",
                                  bufs=1)
                    nc.vector.reciprocal(rt[DH:DH + 1, :], otp[DH:DH + 1, :])
                    rb = psB.tile([DH, TOK], F32, name="rb", tag="rb", bufs=1)
                    nc.tensor.matmul(rb, ones65[DH:DH + 1, :],
                                     rt[DH:DH + 1, :], start=True, stop=True)
                    rbs = pwk.tile([DH, TOK], F16, name="rbs", tag="rbs",
                                   bufs=1)
                    nc.vector.tensor_copy(rbs, rb)
                    nc.vector.tensor_tensor(oT2[:, hp, hh, :],
                                            otp[0:DH, :], rbs, ALU.mult)

                nc.sync.dma_start(out=mt2_s, in_=mt2_d[:])
                with tc.tile_pool(name="psB", bufs=1, space="PSUM") as psB:
                    kq_proj(0, psB)
                    for hp in range(HP):
                        if hp + 1 < HP:
                            kq_proj(hp + 1, psB)
                        if hp == 3:
                            # O-proj phase prefetches, in the DMA-idle window
                            nc.sync.dma_start(out=wo2_sb, in_=wo2_d[:])
                            xq_r = xq_d[:].rearrange("(t p) d -> t p d", p=P)
                            for t in range(NTQ):
                                nc.sync.dma_start(out=xq_sb[t], in_=xq_r[t])
                            nc.sync.dma_start(out=g1_bc,
                                              in_=bcast_row(g1_d[:]))
                            nc.sync.dma_start(out=bb2_bc,
                                              in_=bcast_row(bb2_d[:]))
                            for m in range(2):
                                nc.sync.dma_start(out=w1h_s[m], in_=w1h_d[m])
                                nc.sync.dma_start(out=w1l_s[m], in_=w1l_d[m])
                        if 4 <= hp <= 7:
                            jfs = 4 * (hp - 4)
                            nc.sync.dma_start(
                                out=w2h_sb[:, jfs:jfs + 4, :, :],
                                in_=w2h_d[:, jfs:jfs + 4, :, :])
                        scores_exp(2 * hp, psB)
                        if hp > 0:
                            av_norm(2 * hp - 1, psB)
                        scores_exp(2 * hp + 1, psB)
                        av_norm(2 * hp, psB)
                    av_norm(H - 1, psB)

            # FFN pools open early so w2l/w1 can stream during phase C
            dph = ExitStack()
            pdw = dph.enter_context(tc.tile_pool(name="d_w", bufs=1))
            pdwk = dph.enter_context(tc.tile_pool(name="d_wk", bufs=3))
            g2_bc = pdw.tile([P, D], F16, name="g2_bc")
            be2_bc = pdw.tile([P, D], F16, name="be2_bc")
            nc.sync.dma_start(out=g2_bc, in_=bcast_row(g2_d[:]))
            nc.sync.dma_start(out=be2_bc, in_=bcast_row(be2_d[:]))
            f1h = pdw.tile([P, NJF, 2, TOK], E4, name="f1h")
            f1l = pdw.tile([P, NJF, 2, TOK], E4, name="f1l")
            w2l_sb = pdw.tile([P, NJF, 2, D], E5, name="w2l_sb")

            # ------------- O-projection + LN1 + transpose ------------------
            def ln_normalize(x_tile, wk):
                st = wk.tile([P, 2, 6], F32, name="lnst", tag="lnst")
                xv = x_tile.rearrange("p (s f) -> p s f", f=512)
                for sg in range(2):
                    nc.vector.bn_stats(out=st[:, sg, :], in_=xv[:, sg, :])
                mv = wk.tile([P, 2], F32, name="lnmv", tag="lnmv")
                nc.vector.bn_aggr(out=mv, in_=st)
                sq = wk.tile([P, 1], F32, name="lnsq", tag="lnsq")
                nc.scalar.activation(sq, mv[:, 1:2], AF.Sqrt, bias=eps_t)
                nc.vector.reciprocal(sq, sq)
                nc.gpsimd.tensor_scalar(x_tile, x_tile, mv[:, 0:1], sq,
                                        ALU.subtract, ALU.mult)

            with ExitStack() as cph:
                pcwk = cph.enter_context(tc.tile_pool(name="c_wk", bufs=3))
                pcp = cph.enter_context(tc.tile_pool(name="c_ps", bufs=1,
                                                     space="PSUM"))
                for t in range(NTQ):
                    op = pcp.tile([P, D], F32, name="op", tag="op", bufs=2)
                    for nch in range(4):
                        for hp in range(HP):
                            nc.tensor.matmul(
                                op[:, nch * 256:(nch + 1) * 256],
                                oT2[:, hp, :, t * P:(t + 1) * P],
                                wo2_sb[:, :, hp, nch * 256:(nch + 1) * 256],
                                start=(hp == 0), stop=(hp == HP - 1),
                                perf_mode=DR)
                    s1 = pcwk.tile([P, 1], F32, name="s1", tag="s1")
                    nc.vector.scalar_tensor_tensor(
                        h_t[t], op, 0.0, xq_sb[t], ALU.bypass, ALU.add,
                        accum_out=s1)
                    scr = pcwk.tile([P, D], F16, name="scr", tag="scr")
                    s2 = pcwk.tile([P, 1], F32, name="s2", tag="s2")
                    nc.scalar.activation(scr, h_t[t], AF.Square, accum_out=s2)
                    mean = pcwk.tile([P, 1], F32, name="mean", tag="mean")
                    nc.vector.tensor_scalar(mean, s1, invd_t, None, ALU.mult)
                    var = pcwk.tile([P, 1], F32, name="var", tag="var")
                    nc.vector.scalar_tensor_tensor(var, mean, 0.0, mean,
                                                   ALU.bypass, ALU.mult)
                    nc.vector.scalar_tensor_tensor(var, s2, invd_t, var,
                                                   ALU.mult, ALU.subtract)
                    sq = pcwk.tile([P, 1], F32, name="lnsq", tag="lnsq")
                    nc.scalar.activation(sq, var, AF.Sqrt, bias=eps_t)
                    nc.vector.reciprocal(sq, sq)
                    nc.gpsimd.tensor_scalar(h_t[t], h_t[t], mean, sq,
                                            ALU.subtract, ALU.mult)
                    for half in range(2):
                        tp = pcp.tile([P, 512], F32, name="tp", tag="tp",
                                      bufs=2)
                        for c4 in range(4):
                            c = half * 4 + c4
                            nc.tensor.transpose(
                                tp[:, c4 * P:(c4 + 1) * P],
                                h_t[t][:, c * P:(c + 1) * P], ident)
                        hs = slice(2 * half, 2 * half + 2)
                        ts_ = slice(t * P, (t + 1) * P)
                        tpr = tp[:].rearrange("p (j i c) -> p j i c", j=2, i=2)
                        nc.scalar.activation(hTh[:, hs, :, ts_], tpr, AF.Copy)
                        nc.vector.tensor_tensor(hTl[:, hs, :, ts_], tpr,
                                                hTh[:, hs, :, ts_],
                                                ALU.subtract)
                    # residual term of the final sum: h <- h*g1 + (ln1_b+b2)
                    nc.gpsimd.tensor_tensor(h_t[t], h_t[t], g1_bc, ALU.mult)
                    nc.gpsimd.tensor_tensor(h_t[t], h_t[t], bb2_bc, ALU.add)

        # ---------------- FFN ---------------------------------------------
        with ExitStack() as dph:
            pdw = dph.enter_context(tc.tile_pool(name="d_w", bufs=1))
            pdwk = dph.enter_context(tc.tile_pool(name="d_wk", bufs=3))
            g2_bc = pdw.tile([P, D], F16, name="g2_bc")
            be2_bc = pdw.tile([P, D], F16, name="be2_bc")
            nc.sync.dma_start(out=g2_bc, in_=bcast_row(g2_d[:]))
            nc.sync.dma_start(out=be2_bc, in_=bcast_row(be2_d[:]))
            f1h = pdw.tile([P, NJF, 2, TOK], E4, name="f1h")
            f1l = pdw.tile([P, NJF, 2, TOK], E4, name="f1l")
            w2l_sb = pdw.tile([P, NJF, 2, D], E5, name="w2l_sb")

            with tc.tile_pool(name="d_ps1", bufs=1, space="PSUM") as pd1:
                for m in range(NM1):
                    if m + 2 < NM1:
                        nc.sync.dma_start(out=w1h_s[m + 2], in_=w1h_d[m + 2])
                        nc.sync.dma_start(out=w1l_s[m + 2], in_=w1l_d[m + 2])
                    jf, im = m // 2, m % 2
                    fp = pd1.tile([P, TOK], F32, name="fp", tag="fp", bufs=4)
                    for th in range(2):
                        ts_ = slice(th * 256, (th + 1) * 256)
                        fps = fp[:, ts_]
                        for j in range(NJ):
                            nc.tensor.matmul(fps, w1h_s[m][:, j, :, :],
                                             hTh[:, j, :, ts_],
                                             start=(j == 0), stop=False,
                                             perf_mode=DR)
                            nc.tensor.matmul(fps, w1h_s[m][:, j, :, :],
                                             hTl[:, j, :, ts_],
                                             start=False, stop=False,
                                             perf_mode=DR)
                            nc.tensor.matmul(fps, w1l_s[m][:, j, :, :],
                                             hTh[:, j, :, ts_],
                                             start=False, stop=(j == NJ - 1),
                                             perf_mode=DR)
                        f1f = pdwk.tile([P, 256], F16, name="f1f", tag="f1f")
                        nc.scalar.activation(f1f, fps, AF.Relu,
                                             bias=b1_sb[:, m:m + 1])
                        nc.gpsimd.tensor_copy(f1h[:, jf, im, ts_], f1f)
                        nc.gpsimd.tensor_tensor(f1l[:, jf, im, ts_], f1f,
                                                f1h[:, jf, im, ts_],
                                                ALU.subtract)

            # mm2: w2 fully resident; per-(t, nch) groups run start->stop
            # contiguously so shared PSUM banks see no interleaved starts
            with tc.tile_pool(name="d_ps2", bufs=1, space="PSUM") as pd2:
                for t in range(NTQ):
                    ts_ = slice(t * P, (t + 1) * P)
                    g2p_t = pd2.tile([P, D], F32, name="g2p", tag="g2p",
                                     bufs=2)
                    for nch in range(4):
                        ns = slice(nch * 256, (nch + 1) * 256)
                        for jf in range(NJF):
                            nc.tensor.matmul(g2p_t[:, ns],
                                             f1h[:, jf, :, ts_],
                                             w2h_sb[:, jf, :, ns],
                                             start=(jf == 0), stop=False,
                                             perf_mode=DR)
                            nc.tensor.matmul(g2p_t[:, ns],
                                             f1l[:, jf, :, ts_],
                                             w2h_sb[:, jf, :, ns],
                                             start=False, stop=False,
                                             perf_mode=DR)
                            nc.tensor.matmul(g2p_t[:, ns],
                                             f1h[:, jf, :, ts_],
                                             w2l_sb[:, jf, :, ns],
                                             start=False,
                                             stop=(jf == NJF - 1),
                                             perf_mode=DR)
                    f2 = pdwk.tile([P, D], F32, name="f2", tag="f2", bufs=2)
                    halves = [slice(0, 512), slice(512, D)]
                    for hs in halves:
                        nc.vector.tensor_tensor(f2[:, hs], h_t[t][:, hs],
                                                g2p_t[:, hs], ALU.add)
                    st = pdwk.tile([P, 2, 6], F32, name="lnst", tag="lnst")
                    for sg in range(2):
                        nc.vector.bn_stats(
                            out=st[:, sg, :],
                            in_=f2[:, sg * 512:(sg + 1) * 512])
                    mv = pdwk.tile([P, 2], F32, name="lnmv", tag="lnmv")
                    nc.vector.bn_aggr(out=mv, in_=st)
                    sq = pdwk.tile([P, 1], F32, name="lnsq", tag="lnsq")
                    nc.scalar.activation(sq, mv[:, 1:2], AF.Sqrt, bias=eps_t)
                    nc.vector.reciprocal(sq, sq)
                    for hs in halves:
                        nc.gpsimd.tensor_scalar(f2[:, hs], f2[:, hs],
                                                mv[:, 0:1], sq,
                                                ALU.subtract, ALU.mult)
                        nc.gpsimd.tensor_tensor(f2[:, hs], f2[:, hs],
                                                g2_bc[:, hs], ALU.mult)
                        nc.gpsimd.tensor_tensor(f2[:, hs], f2[:, hs],
                                                be2_bc[:, hs], ALU.add)
                        nc.sync.dma_start(out=out_d[t * P:(t + 1) * P, hs],
                                          in_=f2[:, hs])

    nc.compile()
    return nc


def _pack_dr(w):
    """[D, N] -> [128, D//256, 2, N] (contraction chunk-pairs)."""
    Dd, N = w.shape
    return np.ascontiguousarray(
        w.reshape(Dd // 256, 2, P, N).transpose(2, 0, 1, 3))


def make_in_maps(cfg, inp):
    B, L, D, H, DFF = cfg["B"], cfg["L"], cfg["D"], cfg["H"], cfg["DFF"]
    NCORES = cfg["NCORES"]
    CPB = NCORES // B
    TOK = L // CPB
    KT = L // P
    NM1 = DFF // P
    HPn = H // 2
    f32 = np.float32
    x = np.asarray(inp["x"], f32)
    mask = np.asarray(inp["mask"], bool)
    w = {k: np.asarray(inp[k], f32) for k in
         ("wq", "bq", "wk", "bk", "wv", "bv", "wo", "bo", "w1", "b1",
          "w2", "b2", "ln1_g", "ln1_b", "ln2_g", "ln2_b")}
    bo2 = w["bo"] + w["bv"] @ w["wo"]
    w1s = w["ln1_g"][:, None] * w["w1"]
    b1s = w["b1"] + w["ln1_b"] @ w["w1"]
    bb2 = w["ln1_b"] + w["b2"]

    def hilo(a):
        hi = a.astype(E4NP)
        lo = (a - hi.astype(f32)).astype(E5NP)
        return hi, lo

    w1hf, w1lf = hilo(w1s)
    w2hf, w2lf = hilo(w["w2"])
    w1h = np.stack([_pack_dr(w1hf.astype(f32)[:, m * P:(m + 1) * P])
                    for m in range(NM1)]).astype(E4NP)
    w1l = np.stack([_pack_dr(w1lf.astype(f32)[:, m * P:(m + 1) * P])
                    for m in range(NM1)]).astype(E5NP)
    w2h = _pack_dr(w2hf.astype(f32)).astype(E4NP)
    w2l = _pack_dr(w2lf.astype(f32)).astype(E5NP)
    wo2 = np.ascontiguousarray(
        w["wo"].reshape(HPn, 2, DH, D).transpose(2, 1, 0, 3)).astype(E4NP)
    # wk/wq: per-head-pair slabs [HP, 128, NJ, 2, 128]
    wk2 = np.stack([_pack_dr(w["wk"][:, hp * P:(hp + 1) * P] * 0.125)
                    for hp in range(HPn)]).astype(E4NP)
    wq2 = np.stack([_pack_dr(w["wq"][:, hp * P:(hp + 1) * P])
                    for hp in range(HPn)]).astype(E4NP)
    id2 = np.zeros((DH, 2, P), f32)
    for i in range(2):
        for c in range(DH):
            id2[c, i, c + DH * i] = -192.0
    shared = dict(
        wq2=wq2, wk2=wk2,
        wv2=_pack_dr(w["wv"]).astype(E4NP),
        wo2=wo2, w1h=w1h, w1l=w1l, w2h=w2h, w2l=w2l,
        id2=id2.astype(E4NP),
        bq=w["bq"], bk=w["bk"] * f32(0.125), b1=b1s,
        g1=w["ln1_g"].astype(np.float16), bb2=bb2.astype(np.float16),
        g2=w["ln2_g"].astype(np.float16), be2=w["ln2_b"].astype(np.float16))
    shared = {k: np.ascontiguousarray(v) for k, v in shared.items()}
    in_maps = []
    for c in range(NCORES):
        b, q0 = c // CPB, (c % CPB) * TOK
        xb = x[b]
        m = dict(shared)
        m["xt2"] = _pack_dr(xb.T).astype(E4NP)
        m["xtq2"] = _pack_dr(np.ascontiguousarray(xb[q0:q0 + TOK]).T
                             ).astype(E4NP)
        m["xq"] = np.ascontiguousarray(
            (xb[q0:q0 + TOK] + bo2).astype(np.float16))
        # mt2[c2, kt, i, q] = mask[b, q0+q, kt*128 + i*64 + c2]
        mt = mask[b, q0:q0 + TOK, :].T.astype(f32)  # [L, TOK]
        m["mt2"] = np.ascontiguousarray(
            mt.reshape(KT, 2, DH, TOK).transpose(2, 0, 1, 3)
            .reshape(DH, KT * 2 * TOK)).astype(E4NP)
        in_maps.append(m)
    return in_maps


_NC_CACHE = {}
TRACE = False
LAST_RESULTS = None


def _get_nc(key, cfg):
    if key not in _NC_CACHE:
        _NC_CACHE[key] = build_bass(cfg)
    return _NC_CACHE[key]


def kernel(**inputs):
    global LAST_RESULTS
    from concourse.bass_utils import run_bass_kernel_spmd

    cfg = FULL_CFG
    B, L, D = cfg["B"], cfg["L"], cfg["D"]
    NCORES = cfg["NCORES"]
    CPB = NCORES // B
    TOK = L // CPB
    nc = _get_nc("full", cfg)
    in_maps = make_in_maps(cfg, inputs)
    res = run_bass_kernel_spmd(nc, in_maps, core_ids=list(range(NCORES)),
                               trace=TRACE)
    LAST_RESULTS = res
    out = np.empty((B, L, D), np.float32)
    for c in range(NCORES):
        b, q0 = c // CPB, (c % CPB) * TOK
        out[b, q0:q0 + TOK] = res.results[c]["out"]
    return out


# revision 4
# speedup vs baseline: 1.0240x; 1.0008x over previous
"""Trainium2 Bass kernel for a dense transformer encoder block — fp8 DoubleRow.

Sharding (8 cores): sequence-parallel. Core c handles batch b = c//4 and the
512-token query slice q0 = (c%4)*512; K/V are computed for the full batch on
each core.

Datapath: fp8 e4m3 with DoubleRow matmuls (0.5 cyc/row, 2x contraction packed
into the free dim -> 4x the fp16 MAC rate at K=128). The LxL mask is applied
by an extra DoubleRow matmul (-192*I stationary x binary mask moving)
accumulated straight into the scores PSUM; exp(s-2) then goes PSUM->fp8 on the
ACT engine (the -2 shift keeps e4m3 from overflowing and cancels in the
softmax normalization). Scores contract dh=64 as [32,2] (K/Q folded via
partition-shifted DMAs); AV contracts k-token chunk PAIRS as [128,2] with V
laid out 65-wide per head, read through a 128-wide overlapping AP. The FFN
runs 3-term hi/lo fp8 (ah@bh + al@bh + ah@bl, lo-weights in e5m2).

Schedule: V projection runs first under the initial DMA shadow (V copies on
DVE); then K/Q projection + fold stream one head-pair ahead of a
scores->exp->AV software pipeline (AV lags exp by one head), so the ACT
engine's exp stream - the critical resource - starts early and never waits.
PSUM accumulation groups never interleave with another group's start in the
same bank (hardware lazily zeroes the whole 2KB bank on start_tensor_calc).
"""

import sys
from contextlib import ExitStack

import numpy as np

for _p in ("/opt/trn_rl_repo", "/opt/pypackages"):
    if _p not in sys.path:
        sys.path.append(_p)

import ml_dtypes  # noqa: E402
import concourse.bass as bass  # noqa: E402
import concourse.tile as tile  # noqa: E402
from concourse import bacc, mybir  # noqa: E402
from concourse.masks import make_identity  # noqa: E402

F32 = mybir.dt.float32
F16 = mybir.dt.float16
E4 = mybir.dt.float8e4
E5 = mybir.dt.float8e5
E4NP = ml_dtypes.float8_e4m3
E5NP = ml_dtypes.float8_e5m2
AF = mybir.ActivationFunctionType
ALU = mybir.AluOpType
PM = mybir.MatmulPerfMode
DR = PM.DoubleRow

P = 128
DH = 64
LN_EPS = 1e-5

FULL_CFG = dict(B=2, L=2048, D=1024, H=16, DFF=4096, NCORES=8)


def build_bass(cfg):
    B, L, D, H, DFF = cfg["B"], cfg["L"], cfg["D"], cfg["H"], cfg["DFF"]
    NCORES = cfg["NCORES"]
    CPB = NCORES // B
    TOK = L // CPB              # 512 queries per core
    KT = L // P                 # 16 k-token chunks
    KTP = KT // 2               # 8 chunk pairs
    NJ = D // 256               # 4 D-contraction pairs
    NJF = DFF // 256            # 16 DFF-contraction pairs
    NM1 = DFF // P              # 32 mm1 output groups
    NTQ = TOK // P              # 4 query tiles
    HP = H // 2                 # 8 head pairs
    VW = H * 65 + DH    # vaug row width (65/head + pad; must stay EVEN)

    nc = bacc.Bacc(None, target_bir_lowering=False, debug=False)
    with tile.TileContext(nc) as tc, ExitStack() as top, \
            nc.allow_low_precision(reason="fp8 datapath, fp32 accumulate"):
        dram = top.enter_context(tc.tile_pool(name="dram", bufs=1, space="DRAM"))

        def din(name, shape, dtype=E4):
            return dram.tile(shape, dtype, kind="ExternalInput", name=name,
                             uniquify=False)

        xt2_d = din("xt2", [P, NJ, 2, L])
        xtq2_d = din("xtq2", [P, NJ, 2, TOK])
        wq2_d = din("wq2", [HP, P, NJ, 2, P])
        wk2_d = din("wk2", [HP, P, NJ, 2, P])     # pre-scaled by 1/8
        wv2_d = din("wv2", [P, NJ, 2, D])
        wo2_d = din("wo2", [DH, 2, HP, D])
        w1h_d = din("w1h", [NM1, P, NJ, 2, P])    # g1-folded, hi
        w1l_d = din("w1l", [NM1, P, NJ, 2, P], E5)
        w2h_d = din("w2h", [P, NJF, 2, D])
        w2l_d = din("w2l", [P, NJF, 2, D], E5)
        mt2_d = din("mt2", [DH, KT * 2 * TOK])    # binary mask (1 = masked)
        id2_d = din("id2", [DH, 2, P])            # -192 * packed identity
        xq_d = din("xq", [TOK, D], F16)           # x slice + bo + bv@wo
        bq_d = din("bq", [D], F32)
        bk_d = din("bk", [D], F32)                # pre-scaled by 1/8
        b1_d = din("b1", [DFF], F32)              # b1 + ln1_b @ w1
        g1_d = din("g1", [D], F16)
        bb2_d = din("bb2", [D], F16)              # ln1_b + b2
        g2_d = din("g2", [D], F16)
        be2_d = din("be2", [D], F16)
        out_d = dram.tile([TOK, D], F32, kind="ExternalOutput", name="out",
                          uniquify=False)

        def bcast_row(src_ap):
            return bass.AP(tensor=src_ap.tensor, offset=src_ap.offset,
                           ap=[[0, P]] + [list(a) for a in src_ap.ap])

        const = top.enter_context(tc.tile_pool(name="const", bufs=1))
        ident = const.tile([P, P], F32, name="ident")
        make_identity(nc, ident)
        id2_sb = const.tile([DH, 2, P], E4, name="id2_sb")
        sh_t = const.tile([P, 1], F32, name="sh_t")
        nc.vector.memset(sh_t[:], -2.0)
        eps_t = const.tile([P, 1], F32, name="eps_t")
        nc.vector.memset(eps_t[:], LN_EPS)
        ones65 = const.tile([DH + 1, DH], F16, name="ones65")
        nc.vector.memset(ones65[:], 1.0)
        bq_sb = const.tile([P, HP], F32, name="bq_sb")
        bk_sb = const.tile([P, HP], F32, name="bk_sb")
        b1_sb = const.tile([P, NM1], F32, name="b1_sb")
        invd_t = const.tile([P, 1], F32, name="invd_t")
        nc.vector.memset(invd_t[:], 1.0 / D)

        # cross-phase tiles
        xfer = top.enter_context(tc.tile_pool(name="xfer", bufs=1))
        h_t = [xfer.tile([P, D], F32, name=f"h{t}", tag=f"h{t}")
               for t in range(NTQ)]
        hTh = xfer.tile([P, NJ, 2, TOK], E4, name="hTh")
        hTl = xfer.tile([P, NJ, 2, TOK], E4, name="hTl")
        w2h_sb = xfer.tile([P, NJF, 2, D], E4, name="w2h_sb")
        w1h_s = {}
        w1l_s = {}
        for m in range(NM1):
            w1h_s[m] = xfer.tile([P, NJ, 2, P], E4, name="w1h_s",
                                 tag=f"w1h{m % 3}", bufs=1)
            w1l_s[m] = xfer.tile([P, NJ, 2, P], E5, name="w1l_s",
                                 tag=f"w1l{m % 3}", bufs=1)

        with ExitStack() as mid:
            pxo = mid.enter_context(tc.tile_pool(name="xo_sb", bufs=1))
            oT2 = pxo.tile([DH, HP, 2, TOK], E4, name="oT2")
            wo2_sb = pxo.tile([DH, 2, HP, D], E4, name="wo2_sb")
            xq_sb = [pxo.tile([P, D], F16, name=f"xq{t}", tag=f"xq{t}")
                     for t in range(NTQ)]
            g1_bc = pxo.tile([P, D], F16, name="g1_bc")
            bb2_bc = pxo.tile([P, D], F16, name="bb2_bc")

            # ---------------- attention ------------------------------------
            with ExitStack() as attn:
                pa = attn.enter_context(tc.tile_pool(name="attn_sb", bufs=1))
                pwk = attn.enter_context(tc.tile_pool(name="attn_wk", bufs=2))
                paw = attn.enter_context(tc.tile_pool(name="attn_w", bufs=1))

                mt2_s = pa.tile([DH, KT * 2 * TOK], E4, name="mt2_s")
                mt2_v = mt2_s[:].rearrange("c (k i q) -> c k i q", i=2, q=TOK)
                # vaug: per (ktp, i): 16 heads x 65 (V cols + ones) + pad
                vaug = pa.tile([P, KTP, 2, VW], E4, name="vaug")
                nc.vector.memset(
                    vaug[:, :, :, 0:H * 65].rearrange(
                        "p k i (h w) -> p k i h w", w=65)[:, :, :, :, 64:65],
                    1.0)
                nc.vector.memset(vaug[:, :, :, H * 65:VW], 0.0)

                xt2_s = paw.tile([P, NJ, 2, L], E4, name="xt2_s")
                xtq2_s = paw.tile([P, NJ, 2, TOK], E4, name="xtq2_s")

                # V first: slab0 then wv lead the DMA queue
                with tc.tile_pool(name="attn_v", bufs=1) as pav:
                    wv_s = pav.tile([P, NJ, 2, D], E4, name="wv_s")
                    nc.sync.dma_start(
                        out=xt2_s[:, :, :, 0:512], in_=xt2_d[:, :, :, 0:512])
                    nc.sync.dma_start(out=wv_s, in_=wv2_d[:])
                    for sl in range(1, 4):
                        nc.sync.dma_start(
                            out=xt2_s[:, :, :, sl * 512:(sl + 1) * 512],
                            in_=xt2_d[:, :, :, sl * 512:(sl + 1) * 512])
                    nc.sync.dma_start(out=xtq2_s, in_=xtq2_d[:])
                    nc.sync.dma_start(out=bk_sb,
                                      in_=bk_d[:].rearrange("(c p) -> p c",
                                                            p=P))
                    nc.sync.dma_start(out=bq_sb,
                                      in_=bq_d[:].rearrange("(c p) -> p c",
                                                            p=P))
                    nc.sync.dma_start(out=id2_sb, in_=id2_d[:])
                    nc.sync.dma_start(
                        out=b1_sb, in_=b1_d[:].rearrange("(c p) -> p c", p=P))

                    with tc.tile_pool(name="psV", bufs=1, space="PSUM") as psV:
                        for tc16 in range(KT):
                            vp = psV.tile([P, D], F32, name="vp", tag="vp",
                                          bufs=4)
                            for nch in range(4):
                                for j in range(NJ):
                                    nc.tensor.matmul(
                                        vp[:, nch * 256:(nch + 1) * 256],
                                        xt2_s[:, j, :,
                                              tc16 * P:(tc16 + 1) * P],
                                        wv_s[:, j, :,
                                             nch * 256:(nch + 1) * 256],
                                        start=(j == 0), stop=(j == NJ - 1),
                                        perf_mode=DR)
                            vdst = vaug[:, tc16 // 2, tc16 % 2,
                                        0:H * 65].rearrange(
                                "p (h w) -> p h w", w=65)[:, :, 0:DH]
                            vsrc = vp[:].rearrange("p (h w) -> p h w", w=DH)
                            if tc16 % 2 == 0:
                                nc.scalar.activation(vdst, vsrc, AF.Copy)
                            else:
                                nc.vector.tensor_copy(vdst, vsrc)

                # ---- streamed K/Q projection + scores/exp/AV pipeline ----
                ktq = [pa.tile([DH, 2, L], E4, name="ktq", tag=f"ktq{i % 2}",
                               bufs=1) for i in range(2)]
                qtq = [pa.tile([DH, 2, TOK], E4, name="qtq",
                               tag=f"qtq{i % 2}", bufs=1) for i in range(2)]
                es_t = [pa.tile([P, KTP, 2, TOK], E4, name="es",
                                tag=f"es{i}", bufs=1) for i in range(3)]
                wk_s = {}
                wq_s = {}
                for hp in range(HP):
                    wk_s[hp] = paw.tile([P, NJ, 2, P], E4, name="wk_s",
                                        tag=f"wk{hp % 3}", bufs=1)
                    wq_s[hp] = paw.tile([P, NJ, 2, P], E4, name="wq_s",
                                        tag=f"wq{hp % 3}", bufs=1)
                for hp in range(2):
                    nc.sync.dma_start(out=wk_s[hp], in_=wk2_d[hp])
                    nc.sync.dma_start(out=wq_s[hp], in_=wq2_d[hp])

                def kq_proj(hp, psB):
                    if hp + 2 < HP:
                        nc.sync.dma_start(out=wk_s[hp + 2], in_=wk2_d[hp + 2])
                        nc.sync.dma_start(out=wq_s[hp + 2], in_=wq2_d[hp + 2])
                    kt_t = ktq[hp % 2]
                    kf8 = pwk.tile([P, L], E4, name="kf8", tag="kf8")
                    for tg in range(4):
                        kp = psB.tile([P, 512], F32, name="kp", tag="kp",
                                      bufs=1)
                        for nq in range(2):
                            for j in range(NJ):
                                nc.tensor.matmul(
                                    kp[:, nq * 256:(nq + 1) * 256],
                                    wk_s[hp][:, j, :, :],
                                    xt2_s[:, j, :,
                                          tg * 512 + nq * 256:
                                          tg * 512 + (nq + 1) * 256],
                                    start=(j == 0), stop=(j == NJ - 1),
                                    perf_mode=DR)
                        nc.vector.tensor_scalar(
                            kf8[:, tg * 512:(tg + 1) * 512], kp,
                            bk_sb[:, hp:hp + 1], None, ALU.add)
                    for hh in range(2):
                        for i in range(2):
                            nc.sync.dma_start(
                                out=kt_t[32 * hh:32 * hh + 32, i, :],
                                in_=kf8[64 * hh + 32 * i:
                                        64 * hh + 32 * (i + 1), :])
                    qt_t = qtq[hp % 2]
                    qp = psB.tile([P, TOK], F32, name="qp", tag="kp", bufs=1)
                    for nq in range(2):
                        for j in range(NJ):
                            nc.tensor.matmul(
                                qp[:, nq * 256:(nq + 1) * 256],
                                wq_s[hp][:, j, :, :],
                                xtq2_s[:, j, :, nq * 256:(nq + 1) * 256],
                                start=(j == 0), stop=(j == NJ - 1),
                                perf_mode=DR)
                    qf8 = pwk.tile([P, TOK], E4, name="qf8", tag="qf8")
                    nc.vector.tensor_scalar(qf8, qp, bq_sb[:, hp:hp + 1],
                                            None, ALU.add)
                    for hh in range(2):
                        for i in range(2):
                            nc.sync.dma_start(
                                out=qt_t[32 * hh:32 * hh + 32, i, :],
                                in_=qf8[64 * hh + 32 * i:
                                        64 * hh + 32 * (i + 1), :])

                def scores_exp(h, psB):
                    hp, hh = h // 2, h % 2
                    base = 32 * hh
                    kt_t, qt_t = ktq[hp % 2], qtq[hp % 2]
                    es = es_t[h % 3]
                    for ktp in range(KTP):
                        sp = psB.tile([P, 2, TOK], F32, name="sp",
                                      tag="sp", bufs=2)
                        for i2 in range(2):
                            kt = 2 * ktp + i2
                            for qh in range(2):
                                qs = slice(qh * 256, (qh + 1) * 256)
                                nc.tensor.matmul(
                                    sp[:, i2, qs],
                                    kt_t[base:base + 32, :,
                                         kt * P:(kt + 1) * P],
                                    qt_t[base:base + 32, :, qs],
                                    start=True, stop=False, perf_mode=DR)
                                nc.tensor.matmul(
                                    sp[:, i2, qs], id2_sb[:],
                                    mt2_v[:, kt, :, qs],
                                    start=False, stop=True, perf_mode=DR)
                        nc.scalar.activation(es[:, ktp, :, :], sp, AF.Exp,
                                             bias=sh_t)

                def av_norm(h, psB):
                    hp, hh = h // 2, h % 2
                    es = es_t[h % 3]
                    otp = psB.tile([P, TOK], F32, name="otp", tag="otp",
                                   bufs=2)
                    # qh-outer: each half-bank group runs start->stop without
                    # another group's start in between
                    for qh in range(2):
                        qs = slice(qh * 256, (qh + 1) * 256)
                        for ktp in range(KTP):
                            nc.tensor.matmul(
                                otp[:, qs],
                                vaug[:, ktp, :, 65 * h:65 * h + P],
                                es[:, ktp, :, qs],
                                start=(ktp == 0), stop=(ktp == KTP - 1),
                                perf_mode=DR)
                    rt = pwk.tile([DH + 1, TOK], F16, name="rt", tag="rt",
                                  bufs=1)
                    nc.vector.reciprocal(rt[DH:DH + 1, :], otp[DH:DH + 1, :])
                    rb = psB.tile([DH, TOK], F32, name="rb", tag="rb", bufs=1)
                    nc.tensor.matmul(rb, ones65[DH:DH + 1, :],
                                     rt[DH:DH + 1, :], start=True, stop=True)
                    rbs = pwk.tile([DH, TOK], F16, name="rbs", tag="rbs",
                                   bufs=1)
                    nc.vector.tensor_copy(rbs, rb)
                    nc.vector.tensor_tensor(oT2[:, hp, hh, :],
                                            otp[0:DH, :], rbs, ALU.mult)

                nc.sync.dma_start(out=mt2_s, in_=mt2_d[:])
                with tc.tile_pool(name="psB", bufs=1, space="PSUM") as psB:
                    kq_proj(0, psB)
                    for hp in range(HP):
                        if hp + 1 < HP:
                            kq_proj(hp + 1, psB)
                        if hp == 3:
                            # O-proj phase prefetches, in the DMA-idle window
                            nc.sync.dma_start(out=wo2_sb, in_=wo2_d[:])
                            xq_r = xq_d[:].rearrange("(t p) d -> t p d", p=P)
                            for t in range(NTQ):
                                nc.sync.dma_start(out=xq_sb[t], in_=xq_r[t])
                            nc.sync.dma_start(out=g1_bc,
                                              in_=bcast_row(g1_d[:]))
                            nc.sync.dma_start(out=bb2_bc,
                                              in_=bcast_row(bb2_d[:]))
                            for m in range(2):
                                nc.sync.dma_start(out=w1h_s[m], in_=w1h_d[m])
                                nc.sync.dma_start(out=w1l_s[m], in_=w1l_d[m])
                        if 4 <= hp <= 7:
                            jfs = 4 * (hp - 4)
                            nc.sync.dma_start(
                                out=w2h_sb[:, jfs:jfs + 4, :, :],
                                in_=w2h_d[:, jfs:jfs + 4, :, :])
                        scores_exp(2 * hp, psB)
                        if hp > 0:
                            av_norm(2 * hp - 1, psB)
                        scores_exp(2 * hp + 1, psB)
                        av_norm(2 * hp, psB)
                    av_norm(H - 1, psB)

            # FFN pools open early so w2l/w1 can stream during phase C
            dph = ExitStack()
            pdw = dph.enter_context(tc.tile_pool(name="d_w", bufs=1))
            pdwk = dph.enter_context(tc.tile_pool(name="d_wk", bufs=3))
            g2_bc = pdw.tile([P, D], F16, name="g2_bc")
            be2_bc = pdw.tile([P, D], F16, name="be2_bc")
            nc.sync.dma_start(out=g2_bc, in_=bcast_row(g2_d[:]))
            nc.sync.dma_start(out=be2_bc, in_=bcast_row(be2_d[:]))
            f1h = pdw.tile([P, NJF, 2, TOK], E4, name="f1h")
            f1l = pdw.tile([P, NJF, 2, TOK], E4, name="f1l")
            w2l_sb = pdw.tile([P, NJF, 2, D], E5, name="w2l_sb")

            # ------------- O-projection + LN1 + transpose ------------------
            def ln_normalize(x_tile, wk):
                st = wk.tile([P, 2, 6], F32, name="lnst", tag="lnst")
                xv = x_tile.rearrange("p (s f) -> p s f", f=512)
                for sg in range(2):
                    nc.vector.bn_stats(out=st[:, sg, :], in_=xv[:, sg, :])
                mv = wk.tile([P, 2], F32, name="lnmv", tag="lnmv")
                nc.vector.bn_aggr(out=mv, in_=st)
                sq = wk.tile([P, 1], F32, name="lnsq", tag="lnsq")
                nc.scalar.activation(sq, mv[:, 1:2], AF.Sqrt, bias=eps_t)
                nc.vector.reciprocal(sq, sq)
                nc.gpsimd.tensor_scalar(x_tile, x_tile, mv[:, 0:1], sq,
                                        ALU.subtract, ALU.mult)

            with ExitStack() as cph:
                pcwk = cph.enter_context(tc.tile_pool(name="c_wk", bufs=3))
                pcp = cph.enter_context(tc.tile_pool(name="c_ps", bufs=1,
                                                     space="PSUM"))
                for t in range(NTQ):
                    op = pcp.tile([P, D], F32, name="op", tag="op", bufs=3)
                    for nch in range(4):
                        for hp in range(HP):
                            nc.tensor.matmul(
                                op[:, nch * 256:(nch + 1) * 256],
                                oT2[:, hp, :, t * P:(t + 1) * P],
                                wo2_sb[:, :, hp, nch * 256:(nch + 1) * 256],
                                start=(hp == 0), stop=(hp == HP - 1),
                                perf_mode=DR)
                    s1 = pcwk.tile([P, 1], F32, name="s1", tag="s1")
                    nc.vector.scalar_tensor_tensor(
                        h_t[t], op, 0.0, xq_sb[t], ALU.bypass, ALU.add,
                        accum_out=s1)
                    scr = pcwk.tile([P, D], F16, name="scr", tag="scr")
                    s2 = pcwk.tile([P, 1], F32, name="s2", tag="s2")
                    nc.scalar.activation(scr, h_t[t], AF.Square, accum_out=s2)
                    mean = pcwk.tile([P, 1], F32, name="mean", tag="mean")
                    nc.vector.tensor_scalar(mean, s1, invd_t, None, ALU.mult)
                    var = pcwk.tile([P, 1], F32, name="var", tag="var")
                    nc.vector.scalar_tensor_tensor(var, mean, 0.0, mean,
                                                   ALU.bypass, ALU.mult)
                    nc.vector.scalar_tensor_tensor(var, s2, invd_t, var,
                                                   ALU.mult, ALU.subtract)
                    sq = pcwk.tile([P, 1], F32, name="lnsq", tag="lnsq")
                    nc.scalar.activation(sq, var, AF.Sqrt, bias=eps_t)
                    nc.vector.reciprocal(sq, sq)
                    nc.gpsimd.tensor_scalar(h_t[t], h_t[t], mean, sq,
                                            ALU.subtract, ALU.mult)
                    for half in range(2):
                        tp = pcp.tile([P, 512], F32, name="tp", tag="tp",
                                      bufs=2)
                        for c4 in range(4):
                            c = half * 4 + c4
                            nc.tensor.transpose(
                                tp[:, c4 * P:(c4 + 1) * P],
                                h_t[t][:, c * P:(c + 1) * P], ident)
                        hs = slice(2 * half, 2 * half + 2)
                        ts_ = slice(t * P, (t + 1) * P)
                        tpr = tp[:].rearrange("p (j i c) -> p j i c", j=2, i=2)
                        nc.scalar.activation(hTh[:, hs, :, ts_], tpr, AF.Copy)
                        nc.vector.tensor_tensor(hTl[:, hs, :, ts_], tpr,
                                                hTh[:, hs, :, ts_],
                                                ALU.subtract)
                    # residual term of the final sum: h <- h*g1 + (ln1_b+b2)
                    nc.gpsimd.tensor_tensor(h_t[t], h_t[t], g1_bc, ALU.mult)
                    nc.gpsimd.tensor_tensor(h_t[t], h_t[t], bb2_bc, ALU.add)

        # ---------------- FFN ---------------------------------------------
        with ExitStack() as dph:
            pdw = dph.enter_context(tc.tile_pool(name="d_w", bufs=1))
            pdwk = dph.enter_context(tc.tile_pool(name="d_wk", bufs=3))
            g2_bc = pdw.tile([P, D], F16, name="g2_bc")
            be2_bc = pdw.tile([P, D], F16, name="be2_bc")
            nc.sync.dma_start(out=g2_bc, in_=bcast_row(g2_d[:]))
            nc.sync.dma_start(out=be2_bc, in_=bcast_row(be2_d[:]))
            f1h = pdw.tile([P, NJF, 2, TOK], E4, name="f1h")
            f1l = pdw.tile([P, NJF, 2, TOK], E4, name="f1l")
            w2l_sb = pdw.tile([P, NJF, 2, D], E5, name="w2l_sb")

            with tc.tile_pool(name="d_ps1", bufs=1, space="PSUM") as pd1:
                for m in range(NM1):
                    if m + 2 < NM1:
                        nc.sync.dma_start(out=w1h_s[m + 2], in_=w1h_d[m + 2])
                        nc.sync.dma_start(out=w1l_s[m + 2], in_=w1l_d[m + 2])
                    jf, im = m // 2, m % 2
                    fp = pd1.tile([P, TOK], F32, name="fp", tag="fp", bufs=4)
                    for th in range(2):
                        ts_ = slice(th * 256, (th + 1) * 256)
                        fps = fp[:, ts_]
                        for j in range(NJ):
                            nc.tensor.matmul(fps, w1h_s[m][:, j, :, :],
                                             hTh[:, j, :, ts_],
                                             start=(j == 0), stop=False,
                                             perf_mode=DR)
                            nc.tensor.matmul(fps, w1h_s[m][:, j, :, :],
                                             hTl[:, j, :, ts_],
                                             start=False, stop=False,
                                             perf_mode=DR)
                            nc.tensor.matmul(fps, w1l_s[m][:, j, :, :],
                                             hTh[:, j, :, ts_],
                                             start=False, stop=(j == NJ - 1),
                                             perf_mode=DR)
                        f1f = pdwk.tile([P, 256], F16, name="f1f", tag="f1f")
                        nc.scalar.activation(f1f, fps, AF.Relu,
                                             bias=b1_sb[:, m:m + 1])
                        nc.gpsimd.tensor_copy(f1h[:, jf, im, ts_], f1f)
                        nc.gpsimd.tensor_tensor(f1l[:, jf, im, ts_], f1f,
                                                f1h[:, jf, im, ts_],
                                                ALU.subtract)

            # mm2: w2 fully resident; per-(t, nch) groups run start->stop
            # contiguously so shared PSUM banks see no interleaved starts
            with tc.tile_pool(name="d_ps2", bufs=1, space="PSUM") as pd2:
                for t in range(NTQ):
                    ts_ = slice(t * P, (t + 1) * P)
                    g2p_t = pd2.tile([P, D], F32, name="g2p", tag="g2p",
                                     bufs=2)
                    for nch in range(4):
                        ns = slice(nch * 256, (nch + 1) * 256)
                        for jf in range(NJF):
                            nc.tensor.matmul(g2p_t[:, ns],
                                             f1h[:, jf, :, ts_],
                                             w2h_sb[:, jf, :, ns],
                                             start=(jf == 0), stop=False,
                                             perf_mode=DR)
                            nc.tensor.matmul(g2p_t[:, ns],
                                             f1l[:, jf, :, ts_],
                                             w2h_sb[:, jf, :, ns],
                                             start=False, stop=False,
                                             perf_mode=DR)
                            nc.tensor.matmul(g2p_t[:, ns],
                                             f1h[:, jf, :, ts_],
                                             w2l_sb[:, jf, :, ns],
                                             start=False,
                                             stop=(jf == NJF - 1),
                                             perf_mode=DR)
                    f2 = pdwk.tile([P, D], F32, name="f2", tag="f2", bufs=2)
                    halves = [slice(0, 512), slice(512, D)]
                    for hs in halves:
                        nc.vector.tensor_tensor(f2[:, hs], h_t[t][:, hs],
                                                g2p_t[:, hs], ALU.add)
                    st = pdwk.tile([P, 2, 6], F32, name="lnst", tag="lnst")
                    for sg in range(2):
                        nc.vector.bn_stats(
                            out=st[:, sg, :],
                            in_=f2[:, sg * 512:(sg + 1) * 512])
                    mv = pdwk.tile([P, 2], F32, name="lnmv", tag="lnmv")
                    nc.vector.bn_aggr(out=mv, in_=st)
                    sq = pdwk.tile([P, 1], F32, name="lnsq", tag="lnsq")
                    nc.scalar.activation(sq, mv[:, 1:2], AF.Sqrt, bias=eps_t)
                    nc.vector.reciprocal(sq, sq)
                    for hs in halves:
                        nc.gpsimd.tensor_scalar(f2[:, hs], f2[:, hs],
                                                mv[:, 0:1], sq,
                                                ALU.subtract, ALU.mult)
                        nc.gpsimd.tensor_tensor(f2[:, hs], f2[:, hs],
                                                g2_bc[:, hs], ALU.mult)
                        nc.gpsimd.tensor_tensor(f2[:, hs], f2[:, hs],
                                                be2_bc[:, hs], ALU.add)
                        nc.sync.dma_start(out=out_d[t * P:(t + 1) * P, hs],
                                          in_=f2[:, hs])

    nc.compile()
    return nc


def _pack_dr(w):
    """[D, N] -> [128, D//256, 2, N] (contraction chunk-pairs)."""
    Dd, N = w.shape
    return np.ascontiguousarray(
        w.reshape(Dd // 256, 2, P, N).transpose(2, 0, 1, 3))


def make_in_maps(cfg, inp):
    B, L, D, H, DFF = cfg["B"], cfg["L"], cfg["D"], cfg["H"], cfg["DFF"]
    NCORES = cfg["NCORES"]
    CPB = NCORES // B
    TOK = L // CPB
    KT = L // P
    NM1 = DFF // P
    HPn = H // 2
    f32 = np.float32
    x = np.asarray(inp["x"], f32)
    mask = np.asarray(inp["mask"], bool)
    w = {k: np.asarray(inp[k], f32) for k in
         ("wq", "bq", "wk", "bk", "wv", "bv", "wo", "bo", "w1", "b1",
          "w2", "b2", "ln1_g", "ln1_b", "ln2_g", "ln2_b")}
    bo2 = w["bo"] + w["bv"] @ w["wo"]
    w1s = w["ln1_g"][:, None] * w["w1"]
    b1s = w["b1"] + w["ln1_b"] @ w["w1"]
    bb2 = w["ln1_b"] + w["b2"]

    def hilo(a):
        hi = a.astype(E4NP)
        lo = (a - hi.astype(f32)).astype(E5NP)
        return hi, lo

    w1hf, w1lf = hilo(w1s)
    w2hf, w2lf = hilo(w["w2"])
    w1h = np.stack([_pack_dr(w1hf.astype(f32)[:, m * P:(m + 1) * P])
                    for m in range(NM1)]).astype(E4NP)
    w1l = np.stack([_pack_dr(w1lf.astype(f32)[:, m * P:(m + 1) * P])
                    for m in range(NM1)]).astype(E5NP)
    w2h = _pack_dr(w2hf.astype(f32)).astype(E4NP)
    w2l = _pack_dr(w2lf.astype(f32)).astype(E5NP)
    wo2 = np.ascontiguousarray(
        w["wo"].reshape(HPn, 2, DH, D).transpose(2, 1, 0, 3)).astype(E4NP)
    # wk/wq: per-head-pair slabs [HP, 128, NJ, 2, 128]
    wk2 = np.stack([_pack_dr(w["wk"][:, hp * P:(hp + 1) * P] * 0.125)
                    for hp in range(HPn)]).astype(E4NP)
    wq2 = np.stack([_pack_dr(w["wq"][:, hp * P:(hp + 1) * P])
                    for hp in range(HPn)]).astype(E4NP)
    id2 = np.zeros((DH, 2, P), f32)
    for i in range(2):
        for c in range(DH):
            id2[c, i, c + DH * i] = -192.0
    shared = dict(
        wq2=wq2, wk2=wk2,
        wv2=_pack_dr(w["wv"]).astype(E4NP),
        wo2=wo2, w1h=w1h, w1l=w1l, w2h=w2h, w2l=w2l,
        id2=id2.astype(E4NP),
        bq=w["bq"], bk=w["bk"] * f32(0.125), b1=b1s,
        g1=w["ln1_g"].astype(np.float16), bb2=bb2.astype(np.float16),
        g2=w["ln2_g"].astype(np.float16), be2=w["ln2_b"].astype(np.float16))
    shared = {k: np.ascontiguousarray(v) for k, v in shared.items()}
    in_maps = []
    for c in range(NCORES):
        b, q0 = c // CPB, (c % CPB) * TOK
        xb = x[b]
        m = dict(shared)
        m["xt2"] = _pack_dr(xb.T).astype(E4NP)
        m["xtq2"] = _pack_dr(np.ascontiguousarray(xb[q0:q0 + TOK]).T
                             ).astype(E4NP)
        m["xq"] = np.ascontiguousarray(
            (xb[q0:q0 + TOK] + bo2).astype(np.float16))
        # mt2[c2, kt, i, q] = mask[b, q0+q, kt*128 + i*64 + c2]
        mt = mask[b, q0:q0 + TOK, :].T.astype(f32)  # [L, TOK]
        m["mt2"] = np.ascontiguousarray(
            mt.reshape(KT, 2, DH, TOK).transpose(2, 0, 1, 3)
            .reshape(DH, KT * 2 * TOK)).astype(E4NP)
        in_maps.append(m)
    return in_maps


_NC_CACHE = {}
TRACE = False
LAST_RESULTS = None


def _get_nc(key, cfg):
    if key not in _NC_CACHE:
        _NC_CACHE[key] = build_bass(cfg)
    return _NC_CACHE[key]


def kernel(**inputs):
    global LAST_RESULTS
    from concourse.bass_utils import run_bass_kernel_spmd

    cfg = FULL_CFG
    B, L, D = cfg["B"], cfg["L"], cfg["D"]
    NCORES = cfg["NCORES"]
    CPB = NCORES // B
    TOK = L // CPB
    nc = _get_nc("full", cfg)
    in_maps = make_in_maps(cfg, inputs)
    res = run_bass_kernel_spmd(nc, in_maps, core_ids=list(range(NCORES)),
                               trace=TRACE)
    LAST_RESULTS = res
    out = np.empty((B, L, D), np.float32)
    for c in range(NCORES):
        b, q0 = c // CPB, (c % CPB) * TOK
        out[b, q0:q0 + TOK] = res.results[c]["out"]
    return out


# revision 5
# speedup vs baseline: 1.0326x; 1.0083x over previous
"""Trainium2 Bass kernel for a dense transformer encoder block — fp8 DoubleRow.

Sharding (8 cores): sequence-parallel. Core c handles batch b = c//4 and the
512-token query slice q0 = (c%4)*512; K/V are computed for the full batch on
each core.

Datapath: fp8 e4m3 with DoubleRow matmuls (0.5 cyc/row, 2x contraction packed
into the free dim -> 4x the fp16 MAC rate at K=128). The LxL mask is applied
by an extra DoubleRow matmul (-192*I stationary x binary mask moving)
accumulated straight into the scores PSUM; exp(s-2) then goes PSUM->fp8 on the
ACT engine (the -2 shift keeps e4m3 from overflowing and cancels in the
softmax normalization). Scores contract dh=64 as [32,2] (K/Q folded via
partition-shifted DMAs); AV contracts k-token chunk PAIRS as [128,2] with V
laid out 65-wide per head, read through a 128-wide overlapping AP. The FFN
runs 3-term hi/lo fp8 (ah@bh + al@bh + ah@bl, lo-weights in e5m2).

Schedule: V projection runs first under the initial DMA shadow (V copies on
DVE); then K/Q projection + fold stream one head-pair ahead of a
scores->exp->AV software pipeline (AV lags exp by one head), so the ACT
engine's exp stream - the critical resource - starts early and never waits.
PSUM accumulation groups never interleave with another group's start in the
same bank (hardware lazily zeroes the whole 2KB bank on start_tensor_calc).
"""

import sys
from contextlib import ExitStack

import numpy as np

for _p in ("/opt/trn_rl_repo", "/opt/pypackages"):
    if _p not in sys.path:
        sys.path.append(_p)

import ml_dtypes  # noqa: E402
import concourse.bass as bass  # noqa: E402
import concourse.tile as tile  # noqa: E402
from concourse import bacc, mybir  # noqa: E402
from concourse.masks import make_identity  # noqa: E402

F32 = mybir.dt.float32
F16 = mybir.dt.float16
E4 = mybir.dt.float8e4
E5 = mybir.dt.float8e5
E4NP = ml_dtypes.float8_e4m3
E5NP = ml_dtypes.float8_e5m2
AF = mybir.ActivationFunctionType
ALU = mybir.AluOpType
PM = mybir.MatmulPerfMode
DR = PM.DoubleRow

P = 128
DH = 64
LN_EPS = 1e-5

FULL_CFG = dict(B=2, L=2048, D=1024, H=16, DFF=4096, NCORES=8)


def build_bass(cfg):
    B, L, D, H, DFF = cfg["B"], cfg["L"], cfg["D"], cfg["H"], cfg["DFF"]
    NCORES = cfg["NCORES"]
    CPB = NCORES // B
    TOK = L // CPB              # 512 queries per core
    KT = L // P                 # 16 k-token chunks
    KTP = KT // 2               # 8 chunk pairs
    NJ = D // 256               # 4 D-contraction pairs
    NJF = DFF // 256            # 16 DFF-contraction pairs
    NM1 = DFF // P              # 32 mm1 output groups
    NTQ = TOK // P              # 4 query tiles
    HP = H // 2                 # 8 head pairs
    VW = H * 65 + DH    # vaug row width (65/head + pad; must stay EVEN)

    nc = bacc.Bacc(None, target_bir_lowering=False, debug=False)
    with tile.TileContext(nc) as tc, ExitStack() as top, \
            nc.allow_low_precision(reason="fp8 datapath, fp32 accumulate"):
        dram = top.enter_context(tc.tile_pool(name="dram", bufs=1, space="DRAM"))

        def din(name, shape, dtype=E4):
            return dram.tile(shape, dtype, kind="ExternalInput", name=name,
                             uniquify=False)

        xt2_d = din("xt2", [P, NJ, 2, L])
        xtq2_d = din("xtq2", [P, NJ, 2, TOK])
        wq2_d = din("wq2", [HP, P, NJ, 2, P])
        wk2_d = din("wk2", [HP, P, NJ, 2, P])     # pre-scaled by 1/8
        wv2_d = din("wv2", [P, NJ, 2, D])
        wo2_d = din("wo2", [DH, 2, HP, D])
        w1h_d = din("w1h", [NM1, P, NJ, 2, P])    # g1-folded, hi
        w1l_d = din("w1l", [NM1, P, NJ, 2, P], E5)
        w2h_d = din("w2h", [P, NJF, 2, D])
        w2l_d = din("w2l", [P, NJF, 2, D], E5)
        mt2_d = din("mt2", [DH, KT * 2 * TOK])    # binary mask (1 = masked)
        id2_d = din("id2", [DH, 2, P])            # -192 * packed identity
        xq_d = din("xq", [TOK, D], F16)           # x slice + bo + bv@wo
        bq_d = din("bq", [D], F32)
        bk_d = din("bk", [D], F32)                # pre-scaled by 1/8
        b1_d = din("b1", [DFF], F32)              # b1 + ln1_b @ w1
        g1_d = din("g1", [D], F16)
        bb2_d = din("bb2", [D], F16)              # ln1_b + b2
        g2_d = din("g2", [D], F16)
        be2_d = din("be2", [D], F16)
        out_d = dram.tile([TOK, D], F32, kind="ExternalOutput", name="out",
                          uniquify=False)

        def bcast_row(src_ap):
            return bass.AP(tensor=src_ap.tensor, offset=src_ap.offset,
                           ap=[[0, P]] + [list(a) for a in src_ap.ap])

        const = top.enter_context(tc.tile_pool(name="const", bufs=1))
        ident = const.tile([P, P], F32, name="ident")
        make_identity(nc, ident)
        id2_sb = const.tile([DH, 2, P], E4, name="id2_sb")
        sh_t = const.tile([P, 1], F32, name="sh_t")
        nc.vector.memset(sh_t[:], -2.0)
        eps_t = const.tile([P, 1], F32, name="eps_t")
        nc.vector.memset(eps_t[:], LN_EPS)
        ones65 = const.tile([DH + 1, DH], F16, name="ones65")
        nc.vector.memset(ones65[:], 1.0)
        bq_sb = const.tile([P, HP], F32, name="bq_sb")
        bk_sb = const.tile([P, HP], F32, name="bk_sb")
        b1_sb = const.tile([P, NM1], F32, name="b1_sb")
        invd_t = const.tile([P, 1], F32, name="invd_t")
        nc.vector.memset(invd_t[:], 1.0 / D)

        # cross-phase tiles
        xfer = top.enter_context(tc.tile_pool(name="xfer", bufs=1))
        h_t = [xfer.tile([P, D], F32, name=f"h{t}", tag=f"h{t}")
               for t in range(NTQ)]
        hTh = xfer.tile([P, NJ, 2, TOK], E4, name="hTh")
        hTl = xfer.tile([P, NJ, 2, TOK], E4, name="hTl")
        w2h_sb = xfer.tile([P, NJF, 2, D], E4, name="w2h_sb")
        w1h_s = {}
        w1l_s = {}
        for m in range(NM1):
            w1h_s[m] = xfer.tile([P, NJ, 2, P], E4, name="w1h_s",
                                 tag=f"w1h{m % 3}", bufs=1)
            w1l_s[m] = xfer.tile([P, NJ, 2, P], E5, name="w1l_s",
                                 tag=f"w1l{m % 3}", bufs=1)

        with ExitStack() as mid:
            pxo = mid.enter_context(tc.tile_pool(name="xo_sb", bufs=1))
            oT2 = pxo.tile([DH, HP, 2, TOK], E4, name="oT2")
            wo2_sb = pxo.tile([DH, 2, HP, D], E4, name="wo2_sb")
            xq_sb = [pxo.tile([P, D], F16, name=f"xq{t}", tag=f"xq{t}")
                     for t in range(NTQ)]
            g1_bc = pxo.tile([P, D], F16, name="g1_bc")
            bb2_bc = pxo.tile([P, D], F16, name="bb2_bc")

            # ---------------- attention ------------------------------------
            with ExitStack() as attn:
                pa = attn.enter_context(tc.tile_pool(name="attn_sb", bufs=1))
                pwk = attn.enter_context(tc.tile_pool(name="attn_wk", bufs=2))
                paw = attn.enter_context(tc.tile_pool(name="attn_w", bufs=1))

                mt2_s = pa.tile([DH, KT * 2 * TOK], E4, name="mt2_s")
                mt2_v = mt2_s[:].rearrange("c (k i q) -> c k i q", i=2, q=TOK)
                # vaug: per (ktp, i): 16 heads x 65 (V cols + ones) + pad
                vaug = pa.tile([P, KTP, 2, VW], E4, name="vaug")
                nc.vector.memset(
                    vaug[:, :, :, 0:H * 65].rearrange(
                        "p k i (h w) -> p k i h w", w=65)[:, :, :, :, 64:65],
                    1.0)
                nc.vector.memset(vaug[:, :, :, H * 65:VW], 0.0)

                xt2_s = paw.tile([P, NJ, 2, L], E4, name="xt2_s")
                xtq2_s = paw.tile([P, NJ, 2, TOK], E4, name="xtq2_s")

                # V first: slab0 then wv lead the DMA queue
                with tc.tile_pool(name="attn_v", bufs=1) as pav:
                    wv_s = pav.tile([P, NJ, 2, D], E4, name="wv_s")
                    nc.sync.dma_start(
                        out=xt2_s[:, :, :, 0:512], in_=xt2_d[:, :, :, 0:512])
                    nc.sync.dma_start(out=wv_s, in_=wv2_d[:])
                    for sl in range(1, 4):
                        nc.sync.dma_start(
                            out=xt2_s[:, :, :, sl * 512:(sl + 1) * 512],
                            in_=xt2_d[:, :, :, sl * 512:(sl + 1) * 512])
                    nc.sync.dma_start(out=xtq2_s, in_=xtq2_d[:])
                    nc.sync.dma_start(out=bk_sb,
                                      in_=bk_d[:].rearrange("(c p) -> p c",
                                                            p=P))
                    nc.sync.dma_start(out=bq_sb,
                                      in_=bq_d[:].rearrange("(c p) -> p c",
                                                            p=P))
                    nc.sync.dma_start(out=id2_sb, in_=id2_d[:])
                    nc.sync.dma_start(
                        out=b1_sb, in_=b1_d[:].rearrange("(c p) -> p c", p=P))

                    with tc.tile_pool(name="psV", bufs=1, space="PSUM") as psV:
                        for tc16 in range(KT):
                            vp = psV.tile([P, D], F32, name="vp", tag="vp",
                                          bufs=4)
                            for nch in range(4):
                                for j in range(NJ):
                                    nc.tensor.matmul(
                                        vp[:, nch * 256:(nch + 1) * 256],
                                        xt2_s[:, j, :,
                                              tc16 * P:(tc16 + 1) * P],
                                        wv_s[:, j, :,
                                             nch * 256:(nch + 1) * 256],
                                        start=(j == 0), stop=(j == NJ - 1),
                                        perf_mode=DR)
                            vdst = vaug[:, tc16 // 2, tc16 % 2,
                                        0:H * 65].rearrange(
                                "p (h w) -> p h w", w=65)[:, :, 0:DH]
                            vsrc = vp[:].rearrange("p (h w) -> p h w", w=DH)
                            if tc16 % 2 == 0:
                                nc.scalar.activation(vdst, vsrc, AF.Copy)
                            else:
                                nc.vector.tensor_copy(vdst, vsrc)

                # ---- streamed K/Q projection + scores/exp/AV pipeline ----
                ktq = [pa.tile([DH, 2, L], E4, name="ktq", tag=f"ktq{i % 2}",
                               bufs=1) for i in range(2)]
                qtq = [pa.tile([DH, 2, TOK], E4, name="qtq",
                               tag=f"qtq{i % 2}", bufs=1) for i in range(2)]
                es_t = [pa.tile([P, KTP, 2, TOK], E4, name="es",
                                tag=f"es{i}", bufs=1) for i in range(3)]
                wk_s = {}
                wq_s = {}
                for hp in range(HP):
                    wk_s[hp] = paw.tile([P, NJ, 2, P], E4, name="wk_s",
                                        tag=f"wk{hp % 3}", bufs=1)
                    wq_s[hp] = paw.tile([P, NJ, 2, P], E4, name="wq_s",
                                        tag=f"wq{hp % 3}", bufs=1)
                for hp in range(2):
                    nc.sync.dma_start(out=wk_s[hp], in_=wk2_d[hp])
                    nc.sync.dma_start(out=wq_s[hp], in_=wq2_d[hp])

                def kq_proj(hp, psB):
                    if hp + 2 < HP:
                        nc.sync.dma_start(out=wk_s[hp + 2], in_=wk2_d[hp + 2])
                        nc.sync.dma_start(out=wq_s[hp + 2], in_=wq2_d[hp + 2])
                    kt_t = ktq[hp % 2]
                    kf8 = pwk.tile([P, L], E4, name="kf8", tag="kf8")
                    for tg in range(4):
                        kp = psB.tile([P, 512], F32, name="kp", tag="kp",
                                      bufs=1)
                        for nq in range(2):
                            for j in range(NJ):
                                nc.tensor.matmul(
                                    kp[:, nq * 256:(nq + 1) * 256],
                                    wk_s[hp][:, j, :, :],
                                    xt2_s[:, j, :,
                                          tg * 512 + nq * 256:
                                          tg * 512 + (nq + 1) * 256],
                                    start=(j == 0), stop=(j == NJ - 1),
                                    perf_mode=DR)
                        if hp == 0:
                            nc.scalar.activation(
                                kf8[:, tg * 512:(tg + 1) * 512], kp,
                                AF.Identity, bias=bk_sb[:, hp:hp + 1])
                        else:
                            nc.vector.tensor_scalar(
                                kf8[:, tg * 512:(tg + 1) * 512], kp,
                                bk_sb[:, hp:hp + 1], None, ALU.add)
                    for hh in range(2):
                        for i in range(2):
                            nc.sync.dma_start(
                                out=kt_t[32 * hh:32 * hh + 32, i, :],
                                in_=kf8[64 * hh + 32 * i:
                                        64 * hh + 32 * (i + 1), :])
                    qt_t = qtq[hp % 2]
                    qp = psB.tile([P, TOK], F32, name="qp", tag="kp", bufs=1)
                    for nq in range(2):
                        for j in range(NJ):
                            nc.tensor.matmul(
                                qp[:, nq * 256:(nq + 1) * 256],
                                wq_s[hp][:, j, :, :],
                                xtq2_s[:, j, :, nq * 256:(nq + 1) * 256],
                                start=(j == 0), stop=(j == NJ - 1),
                                perf_mode=DR)
                    qf8 = pwk.tile([P, TOK], E4, name="qf8", tag="qf8")
                    if hp == 0:
                        nc.scalar.activation(qf8, qp, AF.Identity,
                                             bias=bq_sb[:, hp:hp + 1])
                    else:
                        nc.vector.tensor_scalar(qf8, qp, bq_sb[:, hp:hp + 1],
                                                None, ALU.add)
                    for hh in range(2):
                        for i in range(2):
                            nc.sync.dma_start(
                                out=qt_t[32 * hh:32 * hh + 32, i, :],
                                in_=qf8[64 * hh + 32 * i:
                                        64 * hh + 32 * (i + 1), :])

                def scores_exp(h, psB):
                    hp, hh = h // 2, h % 2
                    base = 32 * hh
                    kt_t, qt_t = ktq[hp % 2], qtq[hp % 2]
                    es = es_t[h % 3]
                    for ktp in range(KTP):
                        sp = psB.tile([P, 2, TOK], F32, name="sp",
                                      tag="sp", bufs=2)
                        for i2 in range(2):
                            kt = 2 * ktp + i2
                            for qh in range(2):
                                qs = slice(qh * 256, (qh + 1) * 256)
                                nc.tensor.matmul(
                                    sp[:, i2, qs],
                                    kt_t[base:base + 32, :,
                                         kt * P:(kt + 1) * P],
                                    qt_t[base:base + 32, :, qs],
                                    start=True, stop=False, perf_mode=DR)
                                nc.tensor.matmul(
                                    sp[:, i2, qs], id2_sb[:],
                                    mt2_v[:, kt, :, qs],
                                    start=False, stop=True, perf_mode=DR)
                        nc.scalar.activation(es[:, ktp, :, :], sp, AF.Exp,
                                             bias=sh_t)

                def av_norm(h, psB):
                    hp, hh = h // 2, h % 2
                    es = es_t[h % 3]
                    otp = psB.tile([P, TOK], F32, name="otp", tag="otp",
                                   bufs=2)
                    # qh-outer: each half-bank group runs start->stop without
                    # another group's start in between
                    for qh in range(2):
                        qs = slice(qh * 256, (qh + 1) * 256)
                        for ktp in range(KTP):
                            nc.tensor.matmul(
                                otp[:, qs],
                                vaug[:, ktp, :, 65 * h:65 * h + P],
                                es[:, ktp, :, qs],
                                start=(ktp == 0), stop=(ktp == KTP - 1),
                                perf_mode=DR)
                    rt = pwk.tile([DH + 1, TOK], F16, name="rt", tag="rt",
                                  bufs=1)
                    nc.vector.reciprocal(rt[DH:DH + 1, :], otp[DH:DH + 1, :])
                    rb = psB.tile([DH, TOK], F32, name="rb", tag="rb", bufs=1)
                    nc.tensor.matmul(rb, ones65[DH:DH + 1, :],
                                     rt[DH:DH + 1, :], start=True, stop=True)
                    rbs = pwk.tile([DH, TOK], F16, name="rbs", tag="rbs",
                                   bufs=1)
                    nc.vector.tensor_copy(rbs, rb)
                    nc.vector.tensor_tensor(oT2[:, hp, hh, :],
                                            otp[0:DH, :], rbs, ALU.mult)

                nc.sync.dma_start(out=mt2_s, in_=mt2_d[:])
                with tc.tile_pool(name="psB", bufs=1, space="PSUM") as psB:
                    kq_proj(0, psB)
                    for hp in range(HP):
                        if hp + 1 < HP:
                            kq_proj(hp + 1, psB)
                        if hp == 3:
                            # O-proj phase prefetches, in the DMA-idle window
                            nc.sync.dma_start(out=wo2_sb, in_=wo2_d[:])
                            xq_r = xq_d[:].rearrange("(t p) d -> t p d", p=P)
                            for t in range(NTQ):
                                nc.sync.dma_start(out=xq_sb[t], in_=xq_r[t])
                            nc.sync.dma_start(out=g1_bc,
                                              in_=bcast_row(g1_d[:]))
                            nc.sync.dma_start(out=bb2_bc,
                                              in_=bcast_row(bb2_d[:]))
                            for m in range(2):
                                nc.sync.dma_start(out=w1h_s[m], in_=w1h_d[m])
                                nc.sync.dma_start(out=w1l_s[m], in_=w1l_d[m])
                        if 4 <= hp <= 7:
                            jfs = 4 * (hp - 4)
                            nc.sync.dma_start(
                                out=w2h_sb[:, jfs:jfs + 4, :, :],
                                in_=w2h_d[:, jfs:jfs + 4, :, :])
                        scores_exp(2 * hp, psB)
                        if hp > 0:
                            av_norm(2 * hp - 1, psB)
                        scores_exp(2 * hp + 1, psB)
                        av_norm(2 * hp, psB)
                    av_norm(H - 1, psB)

            # FFN pools open early so w2l/w1 can stream during phase C
            dph = ExitStack()
            pdw = dph.enter_context(tc.tile_pool(name="d_w", bufs=1))
            pdwk = dph.enter_context(tc.tile_pool(name="d_wk", bufs=3))
            g2_bc = pdw.tile([P, D], F16, name="g2_bc")
            be2_bc = pdw.tile([P, D], F16, name="be2_bc")
            nc.sync.dma_start(out=g2_bc, in_=bcast_row(g2_d[:]))
            nc.sync.dma_start(out=be2_bc, in_=bcast_row(be2_d[:]))
            f1h = pdw.tile([P, NJF, 2, TOK], E4, name="f1h")
            f1l = pdw.tile([P, NJF, 2, TOK], E4, name="f1l")
            w2l_sb = pdw.tile([P, NJF, 2, D], E5, name="w2l_sb")

            # ------------- O-projection + LN1 + transpose ------------------
            def ln_normalize(x_tile, wk):
                st = wk.tile([P, 2, 6], F32, name="lnst", tag="lnst")
                xv = x_tile.rearrange("p (s f) -> p s f", f=512)
                for sg in range(2):
                    nc.vector.bn_stats(out=st[:, sg, :], in_=xv[:, sg, :])
                mv = wk.tile([P, 2], F32, name="lnmv", tag="lnmv")
                nc.vector.bn_aggr(out=mv, in_=st)
                sq = wk.tile([P, 1], F32, name="lnsq", tag="lnsq")
                nc.scalar.activation(sq, mv[:, 1:2], AF.Sqrt, bias=eps_t)
                nc.vector.reciprocal(sq, sq)
                nc.gpsimd.tensor_scalar(x_tile, x_tile, mv[:, 0:1], sq,
                                        ALU.subtract, ALU.mult)

            with ExitStack() as cph:
                pcwk = cph.enter_context(tc.tile_pool(name="c_wk", bufs=3))
                pcp = cph.enter_context(tc.tile_pool(name="c_ps", bufs=1,
                                                     space="PSUM"))
                for t in range(NTQ):
                    op = pcp.tile([P, D], F32, name="op", tag="op", bufs=3)
                    for nch in range(4):
                        for hp in range(HP):
                            nc.tensor.matmul(
                                op[:, nch * 256:(nch + 1) * 256],
                                oT2[:, hp, :, t * P:(t + 1) * P],
                                wo2_sb[:, :, hp, nch * 256:(nch + 1) * 256],
                                start=(hp == 0), stop=(hp == HP - 1),
                                perf_mode=DR)
                    s1 = pcwk.tile([P, 1], F32, name="s1", tag="s1")
                    nc.vector.scalar_tensor_tensor(
                        h_t[t], op, 0.0, xq_sb[t], ALU.bypass, ALU.add,
                        accum_out=s1)
                    scr = pcwk.tile([P, D], F16, name="scr", tag="scr")
                    s2 = pcwk.tile([P, 1], F32, name="s2", tag="s2")
                    nc.scalar.activation(scr, h_t[t], AF.Square, accum_out=s2)
                    mean = pcwk.tile([P, 1], F32, name="mean", tag="mean")
                    nc.vector.tensor_scalar(mean, s1, invd_t, None, ALU.mult)
                    var = pcwk.tile([P, 1], F32, name="var", tag="var")
                    nc.vector.scalar_tensor_tensor(var, mean, 0.0, mean,
                                                   ALU.bypass, ALU.mult)
                    nc.vector.scalar_tensor_tensor(var, s2, invd_t, var,
                                                   ALU.mult, ALU.subtract)
                    sq = pcwk.tile([P, 1], F32, name="lnsq", tag="lnsq")
                    nc.scalar.activation(sq, var, AF.Sqrt, bias=eps_t)
                    nc.vector.reciprocal(sq, sq)
                    nc.gpsimd.tensor_scalar(h_t[t], h_t[t], mean, sq,
                                            ALU.subtract, ALU.mult)
                    for half in range(2):
                        tp = pcp.tile([P, 512], F32, name="tp", tag="tp",
                                      bufs=2)
                        for c4 in range(4):
                            c = half * 4 + c4
                            nc.tensor.transpose(
                                tp[:, c4 * P:(c4 + 1) * P],
                                h_t[t][:, c * P:(c + 1) * P], ident)
                        hs = slice(2 * half, 2 * half + 2)
                        ts_ = slice(t * P, (t + 1) * P)
                        tpr = tp[:].rearrange("p (j i c) -> p j i c", j=2, i=2)
                        nc.scalar.activation(hTh[:, hs, :, ts_], tpr, AF.Copy)
                        nc.vector.tensor_tensor(hTl[:, hs, :, ts_], tpr,
                                                hTh[:, hs, :, ts_],
                                                ALU.subtract)
                    # residual term of the final sum: h <- h*g1 + (ln1_b+b2)
                    nc.gpsimd.tensor_tensor(h_t[t], h_t[t], g1_bc, ALU.mult)
                    nc.gpsimd.tensor_tensor(h_t[t], h_t[t], bb2_bc, ALU.add)

        # ---------------- FFN ---------------------------------------------
        with ExitStack() as dph:
            pdw = dph.enter_context(tc.tile_pool(name="d_w", bufs=1))
            pdwk = dph.enter_context(tc.tile_pool(name="d_wk", bufs=3))
            g2_bc = pdw.tile([P, D], F16, name="g2_bc")
            be2_bc = pdw.tile([P, D], F16, name="be2_bc")
            nc.sync.dma_start(out=g2_bc, in_=bcast_row(g2_d[:]))
            nc.sync.dma_start(out=be2_bc, in_=bcast_row(be2_d[:]))
            f1h = pdw.tile([P, NJF, 2, TOK], E4, name="f1h")
            f1l = pdw.tile([P, NJF, 2, TOK], E4, name="f1l")
            w2l_sb = pdw.tile([P, NJF, 2, D], E5, name="w2l_sb")

            with tc.tile_pool(name="d_ps1", bufs=1, space="PSUM") as pd1:
                for m in range(NM1):
                    if m + 2 < NM1:
                        nc.sync.dma_start(out=w1h_s[m + 2], in_=w1h_d[m + 2])
                        nc.sync.dma_start(out=w1l_s[m + 2], in_=w1l_d[m + 2])
                    jf, im = m // 2, m % 2
                    fp = pd1.tile([P, TOK], F32, name="fp", tag="fp", bufs=4)
                    for th in range(2):
                        ts_ = slice(th * 256, (th + 1) * 256)
                        fps = fp[:, ts_]
                        for j in range(NJ):
                            nc.tensor.matmul(fps, w1h_s[m][:, j, :, :],
                                             hTh[:, j, :, ts_],
                                             start=(j == 0), stop=False,
                                             perf_mode=DR)
                            nc.tensor.matmul(fps, w1h_s[m][:, j, :, :],
                                             hTl[:, j, :, ts_],
                                             start=False, stop=False,
                                             perf_mode=DR)
                            nc.tensor.matmul(fps, w1l_s[m][:, j, :, :],
                                             hTh[:, j, :, ts_],
                                             start=False, stop=(j == NJ - 1),
                                             perf_mode=DR)
                        f1f = pdwk.tile([P, 256], F16, name="f1f", tag="f1f")
                        nc.scalar.activation(f1f, fps, AF.Relu,
                                             bias=b1_sb[:, m:m + 1])
                        nc.gpsimd.tensor_copy(f1h[:, jf, im, ts_], f1f)
                        nc.gpsimd.tensor_tensor(f1l[:, jf, im, ts_], f1f,
                                                f1h[:, jf, im, ts_],
                                                ALU.subtract)

            # mm2: w2 fully resident; per-(t, nch) groups run start->stop
            # contiguously so shared PSUM banks see no interleaved starts
            with tc.tile_pool(name="d_ps2", bufs=1, space="PSUM") as pd2:
                for t in range(NTQ):
                    ts_ = slice(t * P, (t + 1) * P)
                    g2p_t = pd2.tile([P, D], F32, name="g2p", tag="g2p",
                                     bufs=2)
                    for nch in range(4):
                        ns = slice(nch * 256, (nch + 1) * 256)
                        for jf in range(NJF):
                            nc.tensor.matmul(g2p_t[:, ns],
                                             f1h[:, jf, :, ts_],
                                             w2h_sb[:, jf, :, ns],
                                             start=(jf == 0), stop=False,
                                             perf_mode=DR)
                            nc.tensor.matmul(g2p_t[:, ns],
                                             f1l[:, jf, :, ts_],
                                             w2h_sb[:, jf, :, ns],
                                             start=False, stop=False,
                                             perf_mode=DR)
                            nc.tensor.matmul(g2p_t[:, ns],
                                             f1h[:, jf, :, ts_],
                                             w2l_sb[:, jf, :, ns],
                                             start=False,
                                             stop=(jf == NJF - 1),
                                             perf_mode=DR)
                    f2 = pdwk.tile([P, D], F32, name="f2", tag="f2", bufs=2)
                    halves = [slice(0, 512), slice(512, D)]
                    for hs in halves:
                        nc.vector.tensor_tensor(f2[:, hs], h_t[t][:, hs],
                                                g2p_t[:, hs], ALU.add)
                    st = pdwk.tile([P, 2, 6], F32, name="lnst", tag="lnst")
                    for sg in range(2):
                        nc.vector.bn_stats(
                            out=st[:, sg, :],
                            in_=f2[:, sg * 512:(sg + 1) * 512])
                    mv = pdwk.tile([P, 2], F32, name="lnmv", tag="lnmv")
                    nc.vector.bn_aggr(out=mv, in_=st)
                    sq = pdwk.tile([P, 1], F32, name="lnsq", tag="lnsq")
                    nc.scalar.activation(sq, mv[:, 1:2], AF.Sqrt, bias=eps_t)
                    nc.vector.reciprocal(sq, sq)
                    for hs in halves:
                        nc.gpsimd.tensor_scalar(f2[:, hs], f2[:, hs],
                                                mv[:, 0:1], sq,
                                                ALU.subtract, ALU.mult)
                        nc.gpsimd.tensor_tensor(f2[:, hs], f2[:, hs],
                                                g2_bc[:, hs], ALU.mult)
                        nc.gpsimd.tensor_tensor(f2[:, hs], f2[:, hs],
                                                be2_bc[:, hs], ALU.add)
                        nc.sync.dma_start(out=out_d[t * P:(t + 1) * P, hs],
                                          in_=f2[:, hs])

    nc.compile()
    return nc


def _pack_dr(w):
    """[D, N] -> [128, D//256, 2, N] (contraction chunk-pairs)."""
    Dd, N = w.shape
    return np.ascontiguousarray(
        w.reshape(Dd // 256, 2, P, N).transpose(2, 0, 1, 3))


def make_in_maps(cfg, inp):
    B, L, D, H, DFF = cfg["B"], cfg["L"], cfg["D"], cfg["H"], cfg["DFF"]
    NCORES = cfg["NCORES"]
    CPB = NCORES // B
    TOK = L // CPB
    KT = L // P
    NM1 = DFF // P
    HPn = H // 2
    f32 = np.float32
    x = np.asarray(inp["x"], f32)
    mask = np.asarray(inp["mask"], bool)
    w = {k: np.asarray(inp[k], f32) for k in
         ("wq", "bq", "wk", "bk", "wv", "bv", "wo", "bo", "w1", "b1",
          "w2", "b2", "ln1_g", "ln1_b", "ln2_g", "ln2_b")}
    bo2 = w["bo"] + w["bv"] @ w["wo"]
    w1s = w["ln1_g"][:, None] * w["w1"]
    b1s = w["b1"] + w["ln1_b"] @ w["w1"]
    bb2 = w["ln1_b"] + w["b2"]

    def hilo(a):
        hi = a.astype(E4NP)
        lo = (a - hi.astype(f32)).astype(E5NP)
        return hi, lo

    w1hf, w1lf = hilo(w1s)
    w2hf, w2lf = hilo(w["w2"])
    w1h = np.stack([_pack_dr(w1hf.astype(f32)[:, m * P:(m + 1) * P])
                    for m in range(NM1)]).astype(E4NP)
    w1l = np.stack([_pack_dr(w1lf.astype(f32)[:, m * P:(m + 1) * P])
                    for m in range(NM1)]).astype(E5NP)
    w2h = _pack_dr(w2hf.astype(f32)).astype(E4NP)
    w2l = _pack_dr(w2lf.astype(f32)).astype(E5NP)
    wo2 = np.ascontiguousarray(
        w["wo"].reshape(HPn, 2, DH, D).transpose(2, 1, 0, 3)).astype(E4NP)
    # wk/wq: per-head-pair slabs [HP, 128, NJ, 2, 128]
    wk2 = np.stack([_pack_dr(w["wk"][:, hp * P:(hp + 1) * P] * 0.125)
                    for hp in range(HPn)]).astype(E4NP)
    wq2 = np.stack([_pack_dr(w["wq"][:, hp * P:(hp + 1) * P])
                    for hp in range(HPn)]).astype(E4NP)
    id2 = np.zeros((DH, 2, P), f32)
    for i in range(2):
        for c in range(DH):
            id2[c, i, c + DH * i] = -192.0
    shared = dict(
        wq2=wq2, wk2=wk2,
        wv2=_pack_dr(w["wv"]).astype(E4NP),
        wo2=wo2, w1h=w1h, w1l=w1l, w2h=w2h, w2l=w2l,
        id2=id2.astype(E4NP),
        bq=w["bq"], bk=w["bk"] * f32(0.125), b1=b1s,
        g1=w["ln1_g"].astype(np.float16), bb2=bb2.astype(np.float16),
        g2=w["ln2_g"].astype(np.float16), be2=w["ln2_b"].astype(np.float16))
    shared = {k: np.ascontiguousarray(v) for k, v in shared.items()}
    in_maps = []
    for c in range(NCORES):
        b, q0 = c // CPB, (c % CPB) * TOK
        xb = x[b]
        m = dict(shared)
        m["xt2"] = _pack_dr(xb.T).astype(E4NP)
        m["xtq2"] = _pack_dr(np.ascontiguousarray(xb[q0:q0 + TOK]).T
                             ).astype(E4NP)
        m["xq"] = np.ascontiguousarray(
            (xb[q0:q0 + TOK] + bo2).astype(np.float16))
        # mt2[c2, kt, i, q] = mask[b, q0+q, kt*128 + i*64 + c2]
        mt = mask[b, q0:q0 + TOK, :].T.astype(f32)  # [L, TOK]
        m["mt2"] = np.ascontiguousarray(
            mt.reshape(KT, 2, DH, TOK).transpose(2, 0, 1, 3)
            .reshape(DH, KT * 2 * TOK)).astype(E4NP)
        in_maps.append(m)
    return in_maps


_NC_CACHE = {}
TRACE = False
LAST_RESULTS = None


def _get_nc(key, cfg):
    if key not in _NC_CACHE:
        _NC_CACHE[key] = build_bass(cfg)
    return _NC_CACHE[key]


def kernel(**inputs):
    global LAST_RESULTS
    from concourse.bass_utils import run_bass_kernel_spmd

    cfg = FULL_CFG
    B, L, D = cfg["B"], cfg["L"], cfg["D"]
    NCORES = cfg["NCORES"]
    CPB = NCORES // B
    TOK = L // CPB
    nc = _get_nc("full", cfg)
    in_maps = make_in_maps(cfg, inputs)
    res = run_bass_kernel_spmd(nc, in_maps, core_ids=list(range(NCORES)),
                               trace=TRACE)
    LAST_RESULTS = res
    out = np.empty((B, L, D), np.float32)
    for c in range(NCORES):
        b, q0 = c // CPB, (c % CPB) * TOK
        out[b, q0:q0 + TOK] = res.results[c]["out"]
    return out


# revision 6
# speedup vs baseline: 1.0395x; 1.0067x over previous
"""Trainium2 Bass kernel for a dense transformer encoder block — fp8 DoubleRow.

Sharding (8 cores): sequence-parallel. Core c handles batch b = c//4 and the
512-token query slice q0 = (c%4)*512; K/V are computed for the full batch on
each core.

Datapath: fp8 e4m3 with DoubleRow matmuls (0.5 cyc/row, 2x contraction packed
into the free dim -> 4x the fp16 MAC rate at K=128). The LxL mask is applied
by an extra DoubleRow matmul (-192*I stationary x binary mask moving)
accumulated straight into the scores PSUM; exp(s-2) then goes PSUM->fp8 on the
ACT engine (the -2 shift keeps e4m3 from overflowing and cancels in the
softmax normalization). Scores contract dh=64 as [32,2] (K/Q folded via
partition-shifted DMAs); AV contracts k-token chunk PAIRS as [128,2] with V
laid out 65-wide per head, read through a 128-wide overlapping AP. The FFN
runs 3-term hi/lo fp8 (ah@bh + al@bh + ah@bl, lo-weights in e5m2).

Schedule: V projection runs first under the initial DMA shadow (V copies on
DVE); then K/Q projection + fold stream one head-pair ahead of a
scores->exp->AV software pipeline (AV lags exp by one head), so the ACT
engine's exp stream - the critical resource - starts early and never waits.
PSUM accumulation groups never interleave with another group's start in the
same bank (hardware lazily zeroes the whole 2KB bank on start_tensor_calc).
"""

import sys
from contextlib import ExitStack

import numpy as np

for _p in ("/opt/trn_rl_repo", "/opt/pypackages"):
    if _p not in sys.path:
        sys.path.append(_p)

import ml_dtypes  # noqa: E402
import concourse.bass as bass  # noqa: E402
import concourse.tile as tile  # noqa: E402
from concourse import bacc, mybir  # noqa: E402
from concourse.masks import make_identity  # noqa: E402

F32 = mybir.dt.float32
F16 = mybir.dt.float16
E4 = mybir.dt.float8e4
E5 = mybir.dt.float8e5
E4NP = ml_dtypes.float8_e4m3
E5NP = ml_dtypes.float8_e5m2
AF = mybir.ActivationFunctionType
ALU = mybir.AluOpType
PM = mybir.MatmulPerfMode
DR = PM.DoubleRow

P = 128
DH = 64
LN_EPS = 1e-5

FULL_CFG = dict(B=2, L=2048, D=1024, H=16, DFF=4096, NCORES=8)


def build_bass(cfg):
    B, L, D, H, DFF = cfg["B"], cfg["L"], cfg["D"], cfg["H"], cfg["DFF"]
    NCORES = cfg["NCORES"]
    CPB = NCORES // B
    TOK = L // CPB              # 512 queries per core
    KT = L // P                 # 16 k-token chunks
    KTP = KT // 2               # 8 chunk pairs
    NJ = D // 256               # 4 D-contraction pairs
    NJF = DFF // 256            # 16 DFF-contraction pairs
    NM1 = DFF // P              # 32 mm1 output groups
    NTQ = TOK // P              # 4 query tiles
    HP = H // 2                 # 8 head pairs
    VW = H * 65 + DH    # vaug row width (65/head + pad; must stay EVEN)

    nc = bacc.Bacc(None, target_bir_lowering=False, debug=False)
    with tile.TileContext(nc) as tc, ExitStack() as top, \
            nc.allow_low_precision(reason="fp8 datapath, fp32 accumulate"):
        dram = top.enter_context(tc.tile_pool(name="dram", bufs=1, space="DRAM"))

        def din(name, shape, dtype=E4):
            return dram.tile(shape, dtype, kind="ExternalInput", name=name,
                             uniquify=False)

        xt2_d = din("xt2", [P, NJ, 2, L])
        xtq2_d = din("xtq2", [P, NJ, 2, TOK])
        wq2_d = din("wq2", [HP, P, NJ, 2, P])
        wk2_d = din("wk2", [HP, P, NJ, 2, P])     # pre-scaled by 1/8
        wv2_d = din("wv2", [P, NJ, 2, D])
        wo2_d = din("wo2", [DH, 2, HP, D])
        w1h_d = din("w1h", [NM1, P, NJ, 2, P])    # g1-folded, hi
        w1l_d = din("w1l", [NM1, P, NJ, 2, P], E5)
        w2h_d = din("w2h", [P, NJF, 2, D])
        w2l_d = din("w2l", [P, NJF, 2, D], E5)
        mt2_d = din("mt2", [DH, KT * 2 * TOK])    # binary mask (1 = masked)
        id2_d = din("id2", [DH, 2, P])            # -192 * packed identity
        xq_d = din("xq", [TOK, D], F16)           # x slice + bo + bv@wo
        bq_d = din("bq", [D], F32)
        bk_d = din("bk", [D], F32)                # pre-scaled by 1/8
        b1_d = din("b1", [DFF], F32)              # b1 + ln1_b @ w1
        g1_d = din("g1", [D], F16)
        bb2_d = din("bb2", [D], F16)              # ln1_b + b2
        g2_d = din("g2", [D], F16)
        be2_d = din("be2", [D], F16)
        out_d = dram.tile([TOK, D], F32, kind="ExternalOutput", name="out",
                          uniquify=False)

        def bcast_row(src_ap):
            return bass.AP(tensor=src_ap.tensor, offset=src_ap.offset,
                           ap=[[0, P]] + [list(a) for a in src_ap.ap])

        const = top.enter_context(tc.tile_pool(name="const", bufs=1))
        ident = const.tile([P, P], F32, name="ident")
        make_identity(nc, ident)
        id2_sb = const.tile([DH, 2, P], E4, name="id2_sb")
        sh_t = const.tile([P, 1], F32, name="sh_t")
        nc.vector.memset(sh_t[:], -2.0)
        eps_t = const.tile([P, 1], F32, name="eps_t")
        nc.vector.memset(eps_t[:], LN_EPS)
        ones65 = const.tile([DH + 1, DH], F16, name="ones65")
        nc.vector.memset(ones65[:], 1.0)
        bq_sb = const.tile([P, HP], F32, name="bq_sb")
        bk_sb = const.tile([P, HP], F32, name="bk_sb")
        b1_sb = const.tile([P, NM1], F32, name="b1_sb")
        invd_t = const.tile([P, 1], F32, name="invd_t")
        nc.vector.memset(invd_t[:], 1.0 / D)

        # cross-phase tiles
        xfer = top.enter_context(tc.tile_pool(name="xfer", bufs=1))
        h_t = [xfer.tile([P, D], F32, name=f"h{t}", tag=f"h{t}")
               for t in range(NTQ)]
        hTh = xfer.tile([P, NJ, 2, TOK], E4, name="hTh")
        hTl = xfer.tile([P, NJ, 2, TOK], E4, name="hTl")
        w2h_sb = xfer.tile([P, NJF, 2, D], E4, name="w2h_sb")
        w1h_s = {}
        w1l_s = {}
        for m in range(NM1):
            w1h_s[m] = xfer.tile([P, NJ, 2, P], E4, name="w1h_s",
                                 tag=f"w1h{m % 3}", bufs=1)
            w1l_s[m] = xfer.tile([P, NJ, 2, P], E5, name="w1l_s",
                                 tag=f"w1l{m % 3}", bufs=1)

        with ExitStack() as mid:
            pxo = mid.enter_context(tc.tile_pool(name="xo_sb", bufs=1))
            oT2 = pxo.tile([DH, HP, 2, TOK], E4, name="oT2")
            wo2_sb = pxo.tile([DH, 2, HP, D], E4, name="wo2_sb")
            xq_sb = [pxo.tile([P, D], F16, name=f"xq{t}", tag=f"xq{t}")
                     for t in range(NTQ)]
            g1_bc = pxo.tile([P, D], F16, name="g1_bc")
            bb2_bc = pxo.tile([P, D], F16, name="bb2_bc")

            # ---------------- attention ------------------------------------
            with ExitStack() as attn:
                pa = attn.enter_context(tc.tile_pool(name="attn_sb", bufs=1))
                pwk = attn.enter_context(tc.tile_pool(name="attn_wk", bufs=2))
                paw = attn.enter_context(tc.tile_pool(name="attn_w", bufs=1))

                mt2_s = pa.tile([DH, KT * 2 * TOK], E4, name="mt2_s")
                mt2_v = mt2_s[:].rearrange("c (k i q) -> c k i q", i=2, q=TOK)
                # vaug: per (ktp, i): 16 heads x 65 (V cols + ones) + pad
                vaug = pa.tile([P, KTP, 2, VW], E4, name="vaug")
                nc.vector.memset(
                    vaug[:, :, :, 0:H * 65].rearrange(
                        "p k i (h w) -> p k i h w", w=65)[:, :, :, :, 64:65],
                    1.0)
                nc.vector.memset(vaug[:, :, :, H * 65:VW], 0.0)

                xt2_s = paw.tile([P, NJ, 2, L], E4, name="xt2_s")
                xtq2_s = paw.tile([P, NJ, 2, TOK], E4, name="xtq2_s")

                ktq = [pa.tile([DH, 2, L], E4, name="ktq", tag=f"ktq{i % 2}",
                               bufs=1) for i in range(2)]
                qtq = [pa.tile([DH, 2, TOK], E4, name="qtq",
                               tag=f"qtq{i % 2}", bufs=1) for i in range(2)]
                es_t = [pa.tile([P, KTP, 2, TOK], E4, name="es",
                                tag=f"es{i}", bufs=1) for i in range(3)]
                wk_s = {}
                wq_s = {}
                for hp in range(HP):
                    wk_s[hp] = paw.tile([P, NJ, 2, P], E4, name="wk_s",
                                        tag=f"wk{hp % 3}", bufs=1)
                    wq_s[hp] = paw.tile([P, NJ, 2, P], E4, name="wq_s",
                                        tag=f"wq{hp % 3}", bufs=1)

                # Q0 first: it only needs wq0+xtq+bq, off the critical chain
                with tc.tile_pool(name="attn_v", bufs=1) as pav:
                    wv_s = pav.tile([P, NJ, 2, D], E4, name="wv_s")
                    nc.sync.dma_start(
                        out=xt2_s[:, :, :, 0:512], in_=xt2_d[:, :, :, 0:512])
                    nc.sync.dma_start(out=wv_s, in_=wv2_d[:])
                    nc.sync.dma_start(out=wq_s[0], in_=wq2_d[0])
                    nc.sync.dma_start(out=xtq2_s, in_=xtq2_d[:])
                    nc.sync.dma_start(out=bq_sb,
                                      in_=bq_d[:].rearrange("(c p) -> p c",
                                                            p=P))
                    for sl in range(1, 4):
                        nc.sync.dma_start(
                            out=xt2_s[:, :, :, sl * 512:(sl + 1) * 512],
                            in_=xt2_d[:, :, :, sl * 512:(sl + 1) * 512])
                    nc.sync.dma_start(out=bk_sb,
                                      in_=bk_d[:].rearrange("(c p) -> p c",
                                                            p=P))
                    nc.sync.dma_start(out=id2_sb, in_=id2_d[:])
                    nc.sync.dma_start(
                        out=b1_sb, in_=b1_d[:].rearrange("(c p) -> p c", p=P))
                    nc.sync.dma_start(out=wk_s[0], in_=wk2_d[0])
                    nc.sync.dma_start(out=wk_s[1], in_=wk2_d[1])
                    nc.sync.dma_start(out=wq_s[1], in_=wq2_d[1])

                    with tc.tile_pool(name="psV", bufs=1, space="PSUM") as psV:
                        for tc16 in range(KT):
                            vp = psV.tile([P, D], F32, name="vp", tag="vp",
                                          bufs=3)
                            for nch in range(4):
                                for j in range(NJ):
                                    nc.tensor.matmul(
                                        vp[:, nch * 256:(nch + 1) * 256],
                                        xt2_s[:, j, :,
                                              tc16 * P:(tc16 + 1) * P],
                                        wv_s[:, j, :,
                                             nch * 256:(nch + 1) * 256],
                                        start=(j == 0), stop=(j == NJ - 1),
                                        perf_mode=DR)
                            vdst = vaug[:, tc16 // 2, tc16 % 2,
                                        0:H * 65].rearrange(
                                "p (h w) -> p h w", w=65)[:, :, 0:DH]
                            vsrc = vp[:].rearrange("p (h w) -> p h w", w=DH)
                            if tc16 % 2 == 0:
                                nc.scalar.activation(vdst, vsrc, AF.Copy)
                            else:
                                nc.vector.tensor_copy(vdst, vsrc)
                        qp0 = psV.tile([P, TOK], F32, name="qp0", tag="qp0",
                                       bufs=1)
                        for nq in range(2):
                            for j in range(NJ):
                                nc.tensor.matmul(
                                    qp0[:, nq * 256:(nq + 1) * 256],
                                    wq_s[0][:, j, :, :],
                                    xtq2_s[:, j, :, nq * 256:(nq + 1) * 256],
                                    start=(j == 0), stop=(j == NJ - 1),
                                    perf_mode=DR)
                        qf8_0 = pwk.tile([P, TOK], E4, name="qf8", tag="qf8")
                        nc.scalar.activation(qf8_0, qp0, AF.Identity,
                                             bias=bq_sb[:, 0:1])
                        for hh in range(2):
                            for i in range(2):
                                nc.sync.dma_start(
                                    out=qtq[0][32 * hh:32 * hh + 32, i, :],
                                    in_=qf8_0[64 * hh + 32 * i:
                                              64 * hh + 32 * (i + 1), :])

                # ---- streamed K/Q projection + scores/exp/AV pipeline ----
                def kq_proj(hp, psB):
                    if hp + 2 < HP:
                        nc.sync.dma_start(out=wk_s[hp + 2], in_=wk2_d[hp + 2])
                        nc.sync.dma_start(out=wq_s[hp + 2], in_=wq2_d[hp + 2])
                    kt_t = ktq[hp % 2]
                    kf8 = pwk.tile([P, L], E4, name="kf8", tag="kf8")
                    for tg in range(4):
                        kp = psB.tile([P, 512], F32, name="kp", tag="kp",
                                      bufs=1)
                        for nq in range(2):
                            for j in range(NJ):
                                nc.tensor.matmul(
                                    kp[:, nq * 256:(nq + 1) * 256],
                                    wk_s[hp][:, j, :, :],
                                    xt2_s[:, j, :,
                                          tg * 512 + nq * 256:
                                          tg * 512 + (nq + 1) * 256],
                                    start=(j == 0), stop=(j == NJ - 1),
                                    perf_mode=DR)
                        if hp == 0:
                            nc.scalar.activation(
                                kf8[:, tg * 512:(tg + 1) * 512], kp,
                                AF.Identity, bias=bk_sb[:, hp:hp + 1])
                        else:
                            nc.vector.tensor_scalar(
                                kf8[:, tg * 512:(tg + 1) * 512], kp,
                                bk_sb[:, hp:hp + 1], None, ALU.add)
                    for hh in range(2):
                        for i in range(2):
                            nc.sync.dma_start(
                                out=kt_t[32 * hh:32 * hh + 32, i, :],
                                in_=kf8[64 * hh + 32 * i:
                                        64 * hh + 32 * (i + 1), :])
                    if hp == 0:
                        return
                    qt_t = qtq[hp % 2]
                    qp = psB.tile([P, TOK], F32, name="qp", tag="kp", bufs=1)
                    for nq in range(2):
                        for j in range(NJ):
                            nc.tensor.matmul(
                                qp[:, nq * 256:(nq + 1) * 256],
                                wq_s[hp][:, j, :, :],
                                xtq2_s[:, j, :, nq * 256:(nq + 1) * 256],
                                start=(j == 0), stop=(j == NJ - 1),
                                perf_mode=DR)
                    qf8 = pwk.tile([P, TOK], E4, name="qf8", tag="qf8")
                    if hp == 0:
                        nc.scalar.activation(qf8, qp, AF.Identity,
                                             bias=bq_sb[:, hp:hp + 1])
                    else:
                        nc.vector.tensor_scalar(qf8, qp, bq_sb[:, hp:hp + 1],
                                                None, ALU.add)
                    for hh in range(2):
                        for i in range(2):
                            nc.sync.dma_start(
                                out=qt_t[32 * hh:32 * hh + 32, i, :],
                                in_=qf8[64 * hh + 32 * i:
                                        64 * hh + 32 * (i + 1), :])

                def scores_exp(h, psB):
                    hp, hh = h // 2, h % 2
                    base = 32 * hh
                    kt_t, qt_t = ktq[hp % 2], qtq[hp % 2]
                    es = es_t[h % 3]
                    for ktp in range(KTP):
                        sp = psB.tile([P, 2, TOK], F32, name="sp",
                                      tag="sp", bufs=2)
                        for i2 in range(2):
                            kt = 2 * ktp + i2
                            for qh in range(2):
                                qs = slice(qh * 256, (qh + 1) * 256)
                                nc.tensor.matmul(
                                    sp[:, i2, qs],
                                    kt_t[base:base + 32, :,
                                         kt * P:(kt + 1) * P],
                                    qt_t[base:base + 32, :, qs],
                                    start=True, stop=False, perf_mode=DR)
                                nc.tensor.matmul(
                                    sp[:, i2, qs], id2_sb[:],
                                    mt2_v[:, kt, :, qs],
                                    start=False, stop=True, perf_mode=DR)
                        nc.scalar.activation(es[:, ktp, :, :], sp, AF.Exp,
                                             bias=sh_t)

                def av_norm(h, psB):
                    hp, hh = h // 2, h % 2
                    es = es_t[h % 3]
                    otp = psB.tile([P, TOK], F32, name="otp", tag="otp",
                                   bufs=2)
                    # qh-outer: each half-bank group runs start->stop without
                    # another group's start in between
                    for qh in range(2):
                        qs = slice(qh * 256, (qh + 1) * 256)
                        for ktp in range(KTP):
                            nc.tensor.matmul(
                                otp[:, qs],
                                vaug[:, ktp, :, 65 * h:65 * h + P],
                                es[:, ktp, :, qs],
                                start=(ktp == 0), stop=(ktp == KTP - 1),
                                perf_mode=DR)
                    rt = pwk.tile([DH + 1, TOK], F16, name="rt", tag="rt",
                                  bufs=1)
                    nc.vector.reciprocal(rt[DH:DH + 1, :], otp[DH:DH + 1, :])
                    rb = psB.tile([DH, TOK], F32, name="rb", tag="rb", bufs=1)
                    nc.tensor.matmul(rb, ones65[DH:DH + 1, :],
                                     rt[DH:DH + 1, :], start=True, stop=True)
                    rbs = pwk.tile([DH, TOK], F16, name="rbs", tag="rbs",
                                   bufs=1)
                    nc.vector.tensor_copy(rbs, rb)
                    nc.vector.tensor_tensor(oT2[:, hp, hh, :],
                                            otp[0:DH, :], rbs, ALU.mult)

                nc.sync.dma_start(out=mt2_s, in_=mt2_d[:])
                with tc.tile_pool(name="psB", bufs=1, space="PSUM") as psB:
                    kq_proj(0, psB)
                    for hp in range(HP):
                        if hp + 1 < HP:
                            kq_proj(hp + 1, psB)
                        if hp == 3:
                            # O-proj phase prefetches, in the DMA-idle window
                            nc.sync.dma_start(out=wo2_sb[:, :, 0:4, :],
                                              in_=wo2_d[:, :, 0:4, :])
                            nc.sync.dma_start(out=wo2_sb[:, :, 4:HP, :],
                                              in_=wo2_d[:, :, 4:HP, :])
                            xq_r = xq_d[:].rearrange("(t p) d -> t p d", p=P)
                            for t in range(NTQ):
                                nc.sync.dma_start(out=xq_sb[t], in_=xq_r[t])
                            nc.sync.dma_start(out=g1_bc,
                                              in_=bcast_row(g1_d[:]))
                            nc.sync.dma_start(out=bb2_bc,
                                              in_=bcast_row(bb2_d[:]))
                            for m in range(2):
                                nc.sync.dma_start(out=w1h_s[m], in_=w1h_d[m])
                                nc.sync.dma_start(out=w1l_s[m], in_=w1l_d[m])
                        if 4 <= hp <= 7:
                            jfs = 4 * (hp - 4)
                            nc.sync.dma_start(
                                out=w2h_sb[:, jfs:jfs + 4, :, :],
                                in_=w2h_d[:, jfs:jfs + 4, :, :])
                        scores_exp(2 * hp, psB)
                        if hp > 0:
                            av_norm(2 * hp - 1, psB)
                        scores_exp(2 * hp + 1, psB)
                        av_norm(2 * hp, psB)
                    av_norm(H - 1, psB)

            # FFN pools open early so w2l/w1 can stream during phase C
            dph = ExitStack()
            pdw = dph.enter_context(tc.tile_pool(name="d_w", bufs=1))
            pdwk = dph.enter_context(tc.tile_pool(name="d_wk", bufs=3))
            g2_bc = pdw.tile([P, D], F16, name="g2_bc")
            be2_bc = pdw.tile([P, D], F16, name="be2_bc")
            nc.sync.dma_start(out=g2_bc, in_=bcast_row(g2_d[:]))
            nc.sync.dma_start(out=be2_bc, in_=bcast_row(be2_d[:]))
            f1h = pdw.tile([P, NJF, 2, TOK], E4, name="f1h")
            f1l = pdw.tile([P, NJF, 2, TOK], E4, name="f1l")
            w2l_sb = pdw.tile([P, NJF, 2, D], E5, name="w2l_sb")

            # ------------- O-projection + LN1 + transpose ------------------
            def ln_normalize(x_tile, wk):
                st = wk.tile([P, 2, 6], F32, name="lnst", tag="lnst")
                xv = x_tile.rearrange("p (s f) -> p s f", f=512)
                for sg in range(2):
                    nc.vector.bn_stats(out=st[:, sg, :], in_=xv[:, sg, :])
                mv = wk.tile([P, 2], F32, name="lnmv", tag="lnmv")
                nc.vector.bn_aggr(out=mv, in_=st)
                sq = wk.tile([P, 1], F32, name="lnsq", tag="lnsq")
                nc.scalar.activation(sq, mv[:, 1:2], AF.Sqrt, bias=eps_t)
                nc.vector.reciprocal(sq, sq)
                nc.gpsimd.tensor_scalar(x_tile, x_tile, mv[:, 0:1], sq,
                                        ALU.subtract, ALU.mult)

            with ExitStack() as cph:
                pcwk = cph.enter_context(tc.tile_pool(name="c_wk", bufs=3))
                pcp = cph.enter_context(tc.tile_pool(name="c_ps", bufs=1,
                                                     space="PSUM"))
                for t in range(NTQ):
                    op = pcp.tile([P, D], F32, name="op", tag="op", bufs=3)
                    for nch in range(4):
                        for hp in range(HP):
                            nc.tensor.matmul(
                                op[:, nch * 256:(nch + 1) * 256],
                                oT2[:, hp, :, t * P:(t + 1) * P],
                                wo2_sb[:, :, hp, nch * 256:(nch + 1) * 256],
                                start=(hp == 0), stop=(hp == HP - 1),
                                perf_mode=DR)
                    s1 = pcwk.tile([P, 1], F32, name="s1", tag="s1")
                    nc.vector.scalar_tensor_tensor(
                        h_t[t], op, 0.0, xq_sb[t], ALU.bypass, ALU.add,
                        accum_out=s1)
                    scr = pcwk.tile([P, D], F16, name="scr", tag="scr")
                    s2 = pcwk.tile([P, 1], F32, name="s2", tag="s2")
                    nc.scalar.activation(scr, h_t[t], AF.Square, accum_out=s2)
                    mean = pcwk.tile([P, 1], F32, name="mean", tag="mean")
                    nc.vector.tensor_scalar(mean, s1, invd_t, None, ALU.mult)
                    var = pcwk.tile([P, 1], F32, name="var", tag="var")
                    nc.vector.scalar_tensor_tensor(var, mean, 0.0, mean,
                                                   ALU.bypass, ALU.mult)
                    nc.vector.scalar_tensor_tensor(var, s2, invd_t, var,
                                                   ALU.mult, ALU.subtract)
                    sq = pcwk.tile([P, 1], F32, name="lnsq", tag="lnsq")
                    nc.scalar.activation(sq, var, AF.Sqrt, bias=eps_t)
                    nc.vector.reciprocal(sq, sq)
                    nc.gpsimd.tensor_scalar(h_t[t], h_t[t], mean, sq,
                                            ALU.subtract, ALU.mult)
                    for half in range(2):
                        tp = pcp.tile([P, 512], F32, name="tp", tag="tp",
                                      bufs=2)
                        for c4 in range(4):
                            c = half * 4 + c4
                            nc.tensor.transpose(
                                tp[:, c4 * P:(c4 + 1) * P],
                                h_t[t][:, c * P:(c + 1) * P], ident)
                        hs = slice(2 * half, 2 * half + 2)
                        ts_ = slice(t * P, (t + 1) * P)
                        tpr = tp[:].rearrange("p (j i c) -> p j i c", j=2, i=2)
                        nc.scalar.activation(hTh[:, hs, :, ts_], tpr, AF.Copy)
                        nc.vector.tensor_tensor(hTl[:, hs, :, ts_], tpr,
                                                hTh[:, hs, :, ts_],
                                                ALU.subtract)
                    # residual term of the final sum: h <- h*g1 + (ln1_b+b2)
                    nc.gpsimd.tensor_tensor(h_t[t], h_t[t], g1_bc, ALU.mult)
                    nc.gpsimd.tensor_tensor(h_t[t], h_t[t], bb2_bc, ALU.add)

        # ---------------- FFN ---------------------------------------------
        with ExitStack() as dph:
            pdw = dph.enter_context(tc.tile_pool(name="d_w", bufs=1))
            pdwk = dph.enter_context(tc.tile_pool(name="d_wk", bufs=3))
            g2_bc = pdw.tile([P, D], F16, name="g2_bc")
            be2_bc = pdw.tile([P, D], F16, name="be2_bc")
            nc.sync.dma_start(out=g2_bc, in_=bcast_row(g2_d[:]))
            nc.sync.dma_start(out=be2_bc, in_=bcast_row(be2_d[:]))
            f1h = pdw.tile([P, NJF, 2, TOK], E4, name="f1h")
            f1l = pdw.tile([P, NJF, 2, TOK], E4, name="f1l")
            w2l_sb = pdw.tile([P, NJF, 2, D], E5, name="w2l_sb")

            with tc.tile_pool(name="d_ps1", bufs=1, space="PSUM") as pd1:
                for m in range(NM1):
                    if m + 2 < NM1:
                        nc.sync.dma_start(out=w1h_s[m + 2], in_=w1h_d[m + 2])
                        nc.sync.dma_start(out=w1l_s[m + 2], in_=w1l_d[m + 2])
                    jf, im = m // 2, m % 2
                    fp = pd1.tile([P, TOK], F32, name="fp", tag="fp", bufs=4)
                    for th in range(2):
                        ts_ = slice(th * 256, (th + 1) * 256)
                        fps = fp[:, ts_]
                        for j in range(NJ):
                            nc.tensor.matmul(fps, w1h_s[m][:, j, :, :],
                                             hTh[:, j, :, ts_],
                                             start=(j == 0), stop=False,
                                             perf_mode=DR)
                            nc.tensor.matmul(fps, w1h_s[m][:, j, :, :],
                                             hTl[:, j, :, ts_],
                                             start=False, stop=False,
                                             perf_mode=DR)
                            nc.tensor.matmul(fps, w1l_s[m][:, j, :, :],
                                             hTh[:, j, :, ts_],
                                             start=False, stop=(j == NJ - 1),
                                             perf_mode=DR)
                        f1f = pdwk.tile([P, 256], F16, name="f1f", tag="f1f")
                        nc.scalar.activation(f1f, fps, AF.Relu,
                                             bias=b1_sb[:, m:m + 1])
                        nc.gpsimd.tensor_copy(f1h[:, jf, im, ts_], f1f)
                        nc.gpsimd.tensor_tensor(f1l[:, jf, im, ts_], f1f,
                                                f1h[:, jf, im, ts_],
                                                ALU.subtract)

            # mm2: w2 fully resident; per-(t, nch) groups run start->stop
            # contiguously so shared PSUM banks see no interleaved starts
            with tc.tile_pool(name="d_ps2", bufs=1, space="PSUM") as pd2:
                for t in range(NTQ):
                    ts_ = slice(t * P, (t + 1) * P)
                    g2p_t = pd2.tile([P, D], F32, name="g2p", tag="g2p",
                                     bufs=2)
                    for nch in range(4):
                        ns = slice(nch * 256, (nch + 1) * 256)
                        for jf in range(NJF):
                            nc.tensor.matmul(g2p_t[:, ns],
                                             f1h[:, jf, :, ts_],
                                             w2h_sb[:, jf, :, ns],
                                             start=(jf == 0), stop=False,
                                             perf_mode=DR)
                            nc.tensor.matmul(g2p_t[:, ns],
                                             f1l[:, jf, :, ts_],
                                             w2h_sb[:, jf, :, ns],
                                             start=False, stop=False,
                                             perf_mode=DR)
                            nc.tensor.matmul(g2p_t[:, ns],
                                             f1h[:, jf, :, ts_],
                                             w2l_sb[:, jf, :, ns],
                                             start=False,
                                             stop=(jf == NJF - 1),
                                             perf_mode=DR)
                    f2 = pdwk.tile([P, D], F32, name="f2", tag="f2", bufs=2)
                    halves = [slice(0, 512), slice(512, D)]
                    for hs in halves:
                        nc.vector.tensor_tensor(f2[:, hs], h_t[t][:, hs],
                                                g2p_t[:, hs], ALU.add)
                    st = pdwk.tile([P, 2, 6], F32, name="lnst", tag="lnst")
                    for sg in range(2):
                        nc.vector.bn_stats(
                            out=st[:, sg, :],
                            in_=f2[:, sg * 512:(sg + 1) * 512])
                    mv = pdwk.tile([P, 2], F32, name="lnmv", tag="lnmv")
                    nc.vector.bn_aggr(out=mv, in_=st)
                    sq = pdwk.tile([P, 1], F32, name="lnsq", tag="lnsq")
                    nc.scalar.activation(sq, mv[:, 1:2], AF.Sqrt, bias=eps_t)
                    nc.vector.reciprocal(sq, sq)
                    for hs in halves:
                        nc.gpsimd.tensor_scalar(f2[:, hs], f2[:, hs],
                                                mv[:, 0:1], sq,
                                                ALU.subtract, ALU.mult)
                        nc.gpsimd.tensor_tensor(f2[:, hs], f2[:, hs],
                                                g2_bc[:, hs], ALU.mult)
                        nc.gpsimd.tensor_tensor(f2[:, hs], f2[:, hs],
                                                be2_bc[:, hs], ALU.add)
                        nc.sync.dma_start(out=out_d[t * P:(t + 1) * P, hs],
                                          in_=f2[:, hs])

    nc.compile()
    return nc


def _pack_dr(w):
    """[D, N] -> [128, D//256, 2, N] (contraction chunk-pairs)."""
    Dd, N = w.shape
    return np.ascontiguousarray(
        w.reshape(Dd // 256, 2, P, N).transpose(2, 0, 1, 3))


def make_in_maps(cfg, inp):
    B, L, D, H, DFF = cfg["B"], cfg["L"], cfg["D"], cfg["H"], cfg["DFF"]
    NCORES = cfg["NCORES"]
    CPB = NCORES // B
    TOK = L // CPB
    KT = L // P
    NM1 = DFF // P
    HPn = H // 2
    f32 = np.float32
    x = np.asarray(inp["x"], f32)
    mask = np.asarray(inp["mask"], bool)
    w = {k: np.asarray(inp[k], f32) for k in
         ("wq", "bq", "wk", "bk", "wv", "bv", "wo", "bo", "w1", "b1",
          "w2", "b2", "ln1_g", "ln1_b", "ln2_g", "ln2_b")}
    bo2 = w["bo"] + w["bv"] @ w["wo"]
    w1s = w["ln1_g"][:, None] * w["w1"]
    b1s = w["b1"] + w["ln1_b"] @ w["w1"]
    bb2 = w["ln1_b"] + w["b2"]

    def hilo(a):
        hi = a.astype(E4NP)
        lo = (a - hi.astype(f32)).astype(E5NP)
        return hi, lo

    w1hf, w1lf = hilo(w1s)
    w2hf, w2lf = hilo(w["w2"])
    w1h = np.stack([_pack_dr(w1hf.astype(f32)[:, m * P:(m + 1) * P])
                    for m in range(NM1)]).astype(E4NP)
    w1l = np.stack([_pack_dr(w1lf.astype(f32)[:, m * P:(m + 1) * P])
                    for m in range(NM1)]).astype(E5NP)
    w2h = _pack_dr(w2hf.astype(f32)).astype(E4NP)
    w2l = _pack_dr(w2lf.astype(f32)).astype(E5NP)
    wo2 = np.ascontiguousarray(
        w["wo"].reshape(HPn, 2, DH, D).transpose(2, 1, 0, 3)).astype(E4NP)
    # wk/wq: per-head-pair slabs [HP, 128, NJ, 2, 128]
    wk2 = np.stack([_pack_dr(w["wk"][:, hp * P:(hp + 1) * P] * 0.125)
                    for hp in range(HPn)]).astype(E4NP)
    wq2 = np.stack([_pack_dr(w["wq"][:, hp * P:(hp + 1) * P])
                    for hp in range(HPn)]).astype(E4NP)
    id2 = np.zeros((DH, 2, P), f32)
    for i in range(2):
        for c in range(DH):
            id2[c, i, c + DH * i] = -192.0
    shared = dict(
        wq2=wq2, wk2=wk2,
        wv2=_pack_dr(w["wv"]).astype(E4NP),
        wo2=wo2, w1h=w1h, w1l=w1l, w2h=w2h, w2l=w2l,
        id2=id2.astype(E4NP),
        bq=w["bq"], bk=w["bk"] * f32(0.125), b1=b1s,
        g1=w["ln1_g"].astype(np.float16), bb2=bb2.astype(np.float16),
        g2=w["ln2_g"].astype(np.float16), be2=w["ln2_b"].astype(np.float16))
    shared = {k: np.ascontiguousarray(v) for k, v in shared.items()}
    in_maps = []
    for c in range(NCORES):
        b, q0 = c // CPB, (c % CPB) * TOK
        xb = x[b]
        m = dict(shared)
        m["xt2"] = _pack_dr(xb.T).astype(E4NP)
        m["xtq2"] = _pack_dr(np.ascontiguousarray(xb[q0:q0 + TOK]).T
                             ).astype(E4NP)
        m["xq"] = np.ascontiguousarray(
            (xb[q0:q0 + TOK] + bo2).astype(np.float16))
        # mt2[c2, kt, i, q] = mask[b, q0+q, kt*128 + i*64 + c2]
        mt = mask[b, q0:q0 + TOK, :].T.astype(f32)  # [L, TOK]
        m["mt2"] = np.ascontiguousarray(
            mt.reshape(KT, 2, DH, TOK).transpose(2, 0, 1, 3)
            .reshape(DH, KT * 2 * TOK)).astype(E4NP)
        in_maps.append(m)
    return in_maps


_NC_CACHE = {}
TRACE = False
LAST_RESULTS = None


def _get_nc(key, cfg):
    if key not in _NC_CACHE:
        _NC_CACHE[key] = build_bass(cfg)
    return _NC_CACHE[key]


def kernel(**inputs):
    global LAST_RESULTS
    from concourse.bass_utils import run_bass_kernel_spmd

    cfg = FULL_CFG
    B, L, D = cfg["B"], cfg["L"], cfg["D"]
    NCORES = cfg["NCORES"]
    CPB = NCORES // B
    TOK = L // CPB
    nc = _get_nc("full", cfg)
    in_maps = make_in_maps(cfg, inputs)
    res = run_bass_kernel_spmd(nc, in_maps, core_ids=list(range(NCORES)),
                               trace=TRACE)
    LAST_RESULTS = res
    out = np.empty((B, L, D), np.float32)
    for c in range(NCORES):
        b, q0 = c // CPB, (c % CPB) * TOK
        out[b, q0:q0 + TOK] = res.results[c]["out"]
    return out
